# revision 23
# baseline (speedup 1.0000x reference)
"""Trainium2 Bass kernel: single-head attention with RoPE and the reference's
multiplicative causal mask (masked logits stay 0 -> exp(0)=1, dense attention).

Sharding: 8 cores = 4 batches x 2 sequence-halves. Each core projects Q/K/V
for its 1024 rows (bf16 matmuls, fp32 PSUM), applies RoPE on-chip, all-gathers
roped K and V within the 2-core pair, then computes attention for its rows.
Output is O^T per core; the host transposes and reassembles.
"""

import sys

for _p in ("/opt/trn_rl_repo", "/root/.axon_site/_ro/trn_rl_repo"):
    if _p not in sys.path:
        sys.path.append(_p)

import math

import ml_dtypes
import numpy as np

BF16 = ml_dtypes.bfloat16

B, S, D = 4, 2048, 2048
NOWN = 1024  # query rows per core
P = 128  # partitions
KD = D // P  # 16 feature chunks
NCJ = S // P  # 16 key chunks
N_CORES = 8
PAIRS = [[0, 1], [2, 3], [4, 5], [6, 7]]
FB = 512  # matmul moving free-dim block
NB = NOWN // FB  # 2 blocks of own rows
SCALE = 1.0 / math.sqrt(S)  # reference scales by sqrt(seq_len), not sqrt(D)

_CACHE = {}


def _build():
    import concourse.bass as bass  # noqa: F401
    import concourse.tile as tile
    from concourse import bacc, mybir

    f32 = mybir.dt.float32
    bf16 = mybir.dt.bfloat16

    nc = bacc.Bacc(
        "TRN2", target_bir_lowering=False, debug=False, num_devices=N_CORES
    )

    x_ext = nc.dram_tensor("x_t", [P, KD, NOWN], bf16, kind="ExternalInput").ap()
    wq_ext = nc.dram_tensor("wq", [KD, P, KD, P], bf16, kind="ExternalInput").ap()
    wk_ext = nc.dram_tensor("wk", [KD, P, KD, P], bf16, kind="ExternalInput").ap()
    wv_ext = nc.dram_tensor("wv", [P, KD, D], bf16, kind="ExternalInput").ap()
    cos_ext = nc.dram_tensor("cos_t", [KD, P, NOWN], bf16, kind="ExternalInput").ap()
    sin_ext = nc.dram_tensor("sin_t", [KD, P, NOWN], bf16, kind="ExternalInput").ap()
    mask_ext = nc.dram_tensor("mask_t", [NCJ, P, NOWN], bf16, kind="ExternalInput").ap()
    out_ext = nc.dram_tensor("out", [D, NOWN], f32, kind="ExternalOutput").ap()

    with tile.TileContext(nc) as tc:
        with (
            tc.tile_pool(name="dram", bufs=1, space="DRAM") as dram,
            tc.tile_pool(name="psum", bufs=6, space="PSUM") as psum,
            tc.tile_pool(name="dnsum", bufs=1, space="PSUM") as dnsum,
            tc.tile_pool(name="persist", bufs=1) as persist,
            tc.tile_pool(name="tmp", bufs=4) as tmp,
            tc.tile_pool(name="csp", bufs=4) as csp,
            tc.tile_pool(name="strm", bufs=8) as strm,
        ):
            kt_local = dram.tile([NCJ // 2, P, KD, P], bf16)
            v_local = dram.tile([NCJ // 2, P, D], bf16)
            kt_g = dram.tile([2, NCJ // 2, P, KD, P], bf16)
            v_g = dram.tile([2, NCJ // 2, P, D], bf16)

            ones_col = persist.tile([P, 1], bf16)
            nc.vector.memset(ones_col, 1.0)
            ones_row = persist.tile([1, P], f32)
            nc.vector.memset(ones_row, 1.0)

            x_pool = tc.alloc_tile_pool(name="x_pool", bufs=1)
            x_sb = x_pool.tile([P, KD, NOWN], bf16)
            # split the load so the first K matmuls can start early
            for kg in range(4):
                nc.sync.dma_start(
                    out=x_sb[:, kg * 4 : (kg + 1) * 4, :],
                    in_=x_ext[:, kg * 4 : (kg + 1) * 4, :],
                )

            # wv lives in its own pool; the load is emitted mid-K-phase so it
            # overlaps K compute without stealing startup DMA bandwidth from x
            wv_pool = tc.alloc_tile_pool(name="wv_pool", bufs=1)
            wv_sb = wv_pool.tile([P, KD, D], bf16)

            def emit_wv_load():
                for kg in range(4):
                    nc.scalar.dma_start(
                        out=wv_sb[:, kg * 4 : (kg + 1) * 4, :],
                        in_=wv_ext[:, kg * 4 : (kg + 1) * 4, :],
                    )

            qt_sb = persist.tile([P, KD, NOWN], bf16)

            def project_rope(w_ext, wpool, out_ap, post=None, hooks=None):
                """Project + RoPE into out_ap(dc, nb); post(dc, nb, ap) after."""
                for dlow in range(KD // 2):
                    if hooks and dlow in hooks:
                        hooks[dlow]()
                    dhigh = dlow + KD // 2
                    cos_t = csp.tile([P, NOWN], bf16, tag="cs")
                    nc.scalar.dma_start(out=cos_t, in_=cos_ext[dlow])
                    sin_t = csp.tile([P, NOWN], bf16, tag="cs")
                    nc.scalar.dma_start(out=sin_t, in_=sin_ext[dlow])
                    cos_h = csp.tile([P, NOWN], bf16, tag="cs")
                    nc.scalar.dma_start(out=cos_h, in_=cos_ext[dhigh])
                    sin_h = csp.tile([P, NOWN], bf16, tag="cs")
                    nc.scalar.dma_start(out=sin_h, in_=sin_ext[dhigh])

                    w_lo = wpool.tile([P, KD, P], bf16, tag="wp")
                    nc.sync.dma_start(out=w_lo, in_=w_ext[dlow])
                    w_hi = wpool.tile([P, KD, P], bf16, tag="wp")
                    nc.sync.dma_start(out=w_hi, in_=w_ext[dhigh])

                    for nb in range(NB):
                        sl = slice(nb * FB, (nb + 1) * FB)
                        ps_lo = psum.tile([P, FB], f32, tag="ps")
                        for k in range(KD):
                            nc.tensor.matmul(
                                ps_lo,
                                lhsT=w_lo[:, k, :],
                                rhs=x_sb[:, k, sl],
                                start=(k == 0),
                                stop=(k == KD - 1),
                            )
                        ps_hi = psum.tile([P, FB], f32, tag="ps")
                        for k in range(KD):
                            nc.tensor.matmul(
                                ps_hi,
                                lhsT=w_hi[:, k, :],
                                rhs=x_sb[:, k, sl],
                                start=(k == 0),
                                stop=(k == KD - 1),
                            )
                        # rope low half: out = lo*cos_l - hi*sin_l
                        t1 = tmp.tile([P, FB], f32, tag="t")
                        nc.vector.tensor_mul(t1, ps_lo, cos_t[:, sl])
                        t2 = tmp.tile([P, FB], f32, tag="t")
                        nc.vector.tensor_mul(t2, ps_hi, sin_t[:, sl])
                        o_lo = out_ap(dlow, nb)
                        nc.vector.tensor_sub(o_lo, t1, t2)
                        if post is not None:
                            post(dlow, nb, o_lo)
                        # rope high half: out = hi*cos_h + lo*sin_h
                        t3 = tmp.tile([P, FB], f32, tag="t")
                        nc.vector.tensor_mul(t3, ps_hi, cos_h[:, sl])
                        t4 = tmp.tile([P, FB], f32, tag="t")
                        nc.vector.tensor_mul(t4, ps_lo, sin_h[:, sl])
                        o_hi = out_ap(dhigh, nb)
                        nc.vector.tensor_add(o_hi, t3, t4)
                        if post is not None:
                            post(dhigh, nb, o_hi)

            # ---- K projection + rope -> kt_local -> AllGather ----
            def k_out(dc, nb):
                t = strm.tile([P, FB], bf16, tag="ro", name=f"kt_{dc}_{nb}")
                return t

            def k_post(dc, nb, t):
                for jj in range(FB // P):
                    nc.gpsimd.dma_start(
                        out=kt_local[nb * 4 + jj][:, dc, :],
                        in_=t[:, jj * P : (jj + 1) * P],
                    )

            with tc.tile_pool(name="wk_pool", bufs=2) as wkp:
                project_rope(wk_ext, wkp, k_out, k_post, hooks={1: emit_wv_load})
            nc.gpsimd.collective_compute(
                "AllGather",
                mybir.AluOpType.bypass,
                replica_groups=PAIRS,
                ins=[kt_local.opt()],
                outs=[kt_g.opt()],
            )

            # ---- V projection (natural layout) -> v_local -> AllGather ----
            for ncc in range(NCJ // 2):
                for wb in range(D // FB):
                    ps_v = psum.tile([P, FB], f32, tag="ps")
                    for k in range(KD):
                        nc.tensor.matmul(
                            ps_v,
                            lhsT=x_sb[:, k, ncc * P : (ncc + 1) * P],
                            rhs=wv_sb[:, k, wb * FB : (wb + 1) * FB],
                            start=(k == 0),
                            stop=(k == KD - 1),
                        )
                    v_t = strm.tile([P, FB], bf16, tag="vo")
                    nc.vector.tensor_copy(v_t, ps_v)
                    nc.gpsimd.dma_start(
                        out=v_local[ncc][:, wb * FB : (wb + 1) * FB], in_=v_t
                    )
            wv_pool.release()
            nc.gpsimd.collective_compute(
                "AllGather",
                mybir.AluOpType.bypass,
                replica_groups=PAIRS,
                ins=[v_local.opt()],
                outs=[v_g.opt()],
            )

            # ---- Q projection + rope (overlaps the collectives) ----
            def q_out(dc, nb):
                return qt_sb[:, dc, nb * FB : (nb + 1) * FB]

            with tc.tile_pool(name="wq_pool", bufs=2) as wqp:
                project_rope(wq_ext, wqp, q_out)
            x_pool.release()

            # ---- Attention ----
            with (
                tc.tile_pool(name="v2_pool", bufs=1) as v2p,
                tc.tile_pool(name="pt_pool", bufs=1) as ptp,
                tc.tile_pool(name="slab", bufs=6) as slab,
                tc.tile_pool(name="mskp", bufs=3) as mskp,
                tc.tile_pool(name="outp", bufs=4) as outp,
                tc.tile_pool(name="smallp", bufs=2) as smallp,
            ):
                v2_sb = v2p.tile([P, NCJ, D], bf16)
                for jc in range(NCJ):
                    nc.gpsimd.dma_start(
                        out=v2_sb[:, jc, :], in_=v_g[jc // 8, jc % 8]
                    )

                pt_sb = [
                    ptp.tile([P, NCJ, FB], bf16, name=f"pt_sb{ib}")
                    for ib in range(NB)
                ]

                # S^T = K @ Q^T; each kt slab is read once and used for both
                # i-blocks; mask loaded full-width per jc
                for jc in range(NCJ):
                    kt_slab = slab.tile([P, KD, P], bf16, tag="slab")
                    nc.sync.dma_start(out=kt_slab, in_=kt_g[jc // 8, jc % 8])
                    msk = mskp.tile([P, NOWN], bf16, tag="m")
                    nc.scalar.dma_start(out=msk, in_=mask_ext[jc])
                    for ib in range(NB):
                        sl = slice(ib * FB, (ib + 1) * FB)
                        ps_s = psum.tile([P, FB], f32, tag="ps")
                        for k in range(KD):
                            nc.tensor.matmul(
                                ps_s,
                                lhsT=kt_slab[:, k, :],
                                rhs=qt_sb[:, k, sl],
                                start=(k == 0),
                                stop=(k == KD - 1),
                            )
                        tm = tmp.tile([P, FB], f32, tag="t")
                        nc.vector.tensor_mul(tm, ps_s, msk[:, sl])
                        nc.scalar.activation(
                            out=pt_sb[ib][:, jc, :],
                            in_=tm,
                            func=mybir.ActivationFunctionType.Exp,
                            scale=SCALE,
                        )

                # denominators + reciprocal broadcasts
                rbs = []
                for ib in range(NB):
                    ps_d = dnsum.tile([1, FB], f32, tag="dn")
                    for jc in range(NCJ):
                        nc.tensor.matmul(
                            ps_d,
                            lhsT=ones_col,
                            rhs=pt_sb[ib][:, jc, :],
                            start=(jc == 0),
                            stop=(jc == NCJ - 1),
                        )
                    recip = smallp.tile([1, FB], f32, tag="rc")
                    nc.vector.reciprocal(recip, ps_d)
                    ps_rb = dnsum.tile([P, FB], f32, tag="rb")
                    nc.tensor.matmul(
                        ps_rb, lhsT=ones_row, rhs=recip, start=True, stop=True
                    )
                    rb = smallp.tile([P, FB], f32, tag="rbs")
                    nc.vector.tensor_copy(rb, ps_rb)
                    rbs.append(rb)

                # O^T = V^T @ P^T, scaled by 1/denom
                for ib in range(NB):
                    rb = rbs[ib]
                    for dc in range(KD):
                        ps_o = psum.tile([P, FB], f32, tag="ps")
                        for jc in range(NCJ):
                            nc.tensor.matmul(
                                ps_o,
                                lhsT=v2_sb[:, jc, dc * P : (dc + 1) * P],
                                rhs=pt_sb[ib][:, jc, :],
                                start=(jc == 0),
                                stop=(jc == NCJ - 1),
                            )
                        o_st = outp.tile([P, FB], f32, tag="o")
                        nc.vector.tensor_mul(o_st, ps_o, rb)
                        nc.gpsimd.dma_start(
                            out=out_ext[
                                dc * P : (dc + 1) * P, ib * FB : (ib + 1) * FB
                            ],
                            in_=o_st,
                        )

    nc.compile()
    return nc


def _prep_inputs(x, cos, sin, Wq, Wk, Wv):
    """Host-side sharding/layout prep. Returns in_maps for 8 cores."""
    x = np.asarray(x, dtype=np.float32)
    cos = np.asarray(cos, dtype=np.float32)
    sin = np.asarray(sin, dtype=np.float32)

    def w_panels(w):
        # W.T [din, dout] -> [dc, p_din, k_din, c_dout] with d = k*128+p
        wt = np.ascontiguousarray(np.asarray(w, dtype=np.float32).T).astype(BF16)
        return np.ascontiguousarray(
            wt.reshape(KD, P, KD, P).transpose(2, 1, 0, 3)
        )

    wq_p = w_panels(Wq)
    wk_p = w_panels(Wk)
    # Wv.T [din, dout] -> [p, k, dout]
    wv_p = np.ascontiguousarray(
        np.asarray(Wv, dtype=np.float32).T.astype(BF16).reshape(KD, P, D).transpose(1, 0, 2)
    )

    in_maps = []
    for c in range(N_CORES):
        b, h = divmod(c, 2)
        rows = slice(h * NOWN, (h + 1) * NOWN)
        xt = np.ascontiguousarray(
            x[b, rows, :].T.astype(BF16).reshape(KD, P, NOWN).transpose(1, 0, 2)
        )
        cos_t = np.ascontiguousarray(cos[rows].T.astype(BF16).reshape(KD, P, NOWN))
        sin_t = np.ascontiguousarray(sin[rows].T.astype(BF16).reshape(KD, P, NOWN))
        j = np.arange(S, dtype=np.int64)[:, None]
        i = (np.arange(NOWN, dtype=np.int64) + h * NOWN)[None, :]
        mask_t = (j <= i).astype(BF16).reshape(NCJ, P, NOWN)
        in_maps.append(
            {
                "x_t": xt,
                "wq": wq_p,
                "wk": wk_p,
                "wv": wv_p,
                "cos_t": cos_t,
                "sin_t": sin_t,
                "mask_t": mask_t,
            }
        )
    return in_maps


def _run(in_maps, trace=False, tmpdir=None):
    from concourse.bass_utils import run_bass_kernel_spmd

    if "nc" not in _CACHE:
        _CACHE["nc"] = _build()
    nc = _CACHE["nc"]
    return run_bass_kernel_spmd(
        nc, in_maps, list(range(N_CORES)), trace=trace, tmpdir=tmpdir
    )


def kernel(x, cos, sin, Wq, Wk, Wv):
    in_maps = _prep_inputs(x, cos, sin, Wq, Wk, Wv)
    res = _run(in_maps, trace=False)
    out = np.empty((B, S, D), dtype=np.float32)
    for c in range(N_CORES):
        b, h = divmod(c, 2)
        out[b, h * NOWN : (h + 1) * NOWN, :] = res.results[c]["out"].T
    return out


# revision 31
# speedup vs baseline: 1.0380x; 1.0380x over previous
"""Trainium2 Bass kernel: single-head attention with RoPE and the reference's
multiplicative causal mask (masked logits stay 0 -> exp(0)=1, dense attention).

Sharding: 8 cores = 4 batches x 2 sequence-halves. Each core projects Q/K/V
for its 1024 rows (bf16 matmuls, fp32 PSUM), applies RoPE on-chip, all-gathers
roped K and V within the 2-core pair, then computes attention for its rows.
Output is O^T per core; the host transposes and reassembles.
"""

import sys

for _p in ("/opt/trn_rl_repo", "/root/.axon_site/_ro/trn_rl_repo"):
    if _p not in sys.path:
        sys.path.append(_p)

import math

import ml_dtypes
import numpy as np

BF16 = ml_dtypes.bfloat16

B, S, D = 4, 2048, 2048
NOWN = 1024  # query rows per core
P = 128  # partitions
KD = D // P  # 16 feature chunks
NCJ = S // P  # 16 key chunks
N_CORES = 8
PAIRS = [[0, 1], [2, 3], [4, 5], [6, 7]]
FB = 512  # matmul moving free-dim block
NB = NOWN // FB  # 2 blocks of own rows
SCALE = 1.0 / math.sqrt(S)  # reference scales by sqrt(seq_len), not sqrt(D)

_CACHE = {}


def _build():
    import concourse.bass as bass  # noqa: F401
    import concourse.tile as tile
    from concourse import bacc, mybir

    f32 = mybir.dt.float32
    bf16 = mybir.dt.bfloat16

    nc = bacc.Bacc(
        "TRN2", target_bir_lowering=False, debug=False, num_devices=N_CORES
    )

    x_ext = nc.dram_tensor("x_t", [P, KD, NOWN], bf16, kind="ExternalInput").ap()
    wq_ext = nc.dram_tensor("wq", [KD, P, KD, P], bf16, kind="ExternalInput").ap()
    wk_ext = nc.dram_tensor("wk", [KD, P, KD, P], bf16, kind="ExternalInput").ap()
    wv_ext = nc.dram_tensor("wv", [P, KD, D], bf16, kind="ExternalInput").ap()
    cos_ext = nc.dram_tensor("cos_t", [KD, P, NOWN], bf16, kind="ExternalInput").ap()
    sin_ext = nc.dram_tensor("sin_t", [KD, P, NOWN], bf16, kind="ExternalInput").ap()
    mask_ext = nc.dram_tensor("mask_t", [NCJ, P, NOWN], bf16, kind="ExternalInput").ap()
    out_ext = nc.dram_tensor("out", [D, NOWN], f32, kind="ExternalOutput").ap()

    with tile.TileContext(nc) as tc:
        with (
            tc.tile_pool(name="dram", bufs=1, space="DRAM") as dram,
            tc.tile_pool(name="psum", bufs=6, space="PSUM") as psum,
            tc.tile_pool(name="dnsum", bufs=1, space="PSUM") as dnsum,
            tc.tile_pool(name="persist", bufs=1) as persist,
            tc.tile_pool(name="tmp", bufs=4) as tmp,
            tc.tile_pool(name="csp", bufs=4) as csp,
            tc.tile_pool(name="strm", bufs=8) as strm,
        ):
            kt_local = dram.tile([NCJ // 2, P, KD, P], bf16)
            v_local = dram.tile([NCJ // 2, P, D], bf16)
            # gathered tensors, split in halves so each 2MB gather can launch
            # as soon as its half is produced (pipelines with compute)
            kt_ga = dram.tile([2, 4, P, KD, P], bf16)
            kt_gb = dram.tile([2, 4, P, KD, P], bf16)
            v_ga = dram.tile([2, 4, P, D], bf16)
            v_gb = dram.tile([2, 4, P, D], bf16)

            def kt_g(jc):
                h2, jcl = jc // 8, jc % 8
                return (kt_ga if jcl < 4 else kt_gb)[h2, jcl % 4]

            def v_g(jc):
                h2, jcl = jc // 8, jc % 8
                return (v_ga if jcl < 4 else v_gb)[h2, jcl % 4]

            ones_col = persist.tile([P, 1], bf16)
            nc.vector.memset(ones_col, 1.0)
            ones_row = persist.tile([1, P], f32)
            nc.vector.memset(ones_row, 1.0)

            x_pool = tc.alloc_tile_pool(name="x_pool", bufs=1)
            x_sb = x_pool.tile([P, KD, NOWN], bf16)
            # split the load across engines/queues so the K matmuls start early
            x_dma_engines = [nc.sync, nc.scalar, nc.gpsimd, nc.sync]
            for kg in range(8):
                x_dma_engines[kg % 4].dma_start(
                    out=x_sb[:, kg * 2 : (kg + 1) * 2, :],
                    in_=x_ext[:, kg * 2 : (kg + 1) * 2, :],
                )

            # wv lives in its own pool; the load is emitted mid-K-phase so it
            # overlaps K compute without stealing startup DMA bandwidth from x
            wv_pool = tc.alloc_tile_pool(name="wv_pool", bufs=1)
            wv_sb = wv_pool.tile([P, KD, D], bf16)

            def emit_wv_load():
                for kg in range(4):
                    nc.scalar.dma_start(
                        out=wv_sb[:, kg * 4 : (kg + 1) * 4, :],
                        in_=wv_ext[:, kg * 4 : (kg + 1) * 4, :],
                    )

            qt_sb = persist.tile([P, KD, NOWN], bf16)

            def rope_pair(wpool, w_ext, dlow, nb, cs_tiles, out_ap, post):
                """One (dlow, nb) unit: two projections + rope."""
                dhigh = dlow + KD // 2
                sl = slice(nb * FB, (nb + 1) * FB)
                cos_t, sin_t, cos_h, sin_h = cs_tiles
                w_lo = wpool.tile([P, KD, P], bf16, tag="wp", name=f"wlo{dlow}{nb}")
                nc.sync.dma_start(out=w_lo, in_=w_ext[dlow])
                w_hi = wpool.tile([P, KD, P], bf16, tag="wp", name=f"whi{dlow}{nb}")
                nc.sync.dma_start(out=w_hi, in_=w_ext[dhigh])
                ps_lo = psum.tile([P, FB], f32, tag="ps", name=f"plo{dlow}{nb}")
                for k in range(KD):
                    nc.tensor.matmul(
                        ps_lo,
                        lhsT=w_lo[:, k, :],
                        rhs=x_sb[:, k, sl],
                        start=(k == 0),
                        stop=(k == KD - 1),
                    )
                ps_hi = psum.tile([P, FB], f32, tag="ps", name=f"phi{dlow}{nb}")
                for k in range(KD):
                    nc.tensor.matmul(
                        ps_hi,
                        lhsT=w_hi[:, k, :],
                        rhs=x_sb[:, k, sl],
                        start=(k == 0),
                        stop=(k == KD - 1),
                    )
                # rope low half: out = lo*cos_l - hi*sin_l
                t1 = tmp.tile([P, FB], f32, tag="t", name=f"t1{dlow}{nb}")
                nc.vector.tensor_mul(t1, ps_lo, cos_t)
                t2 = tmp.tile([P, FB], f32, tag="t", name=f"t2{dlow}{nb}")
                nc.vector.tensor_mul(t2, ps_hi, sin_t)
                o_lo = out_ap(dlow, nb)
                nc.vector.tensor_sub(o_lo, t1, t2)
                if post is not None:
                    post(dlow, nb, o_lo)
                # rope high half: out = hi*cos_h + lo*sin_h
                t3 = tmp.tile([P, FB], f32, tag="t", name=f"t3{dlow}{nb}")
                nc.vector.tensor_mul(t3, ps_hi, cos_h)
                t4 = tmp.tile([P, FB], f32, tag="t", name=f"t4{dlow}{nb}")
                nc.vector.tensor_mul(t4, ps_lo, sin_h)
                o_hi = out_ap(dhigh, nb)
                nc.vector.tensor_add(o_hi, t3, t4)
                if post is not None:
                    post(dhigh, nb, o_hi)

            def cs_load(dlow, sl):
                dhigh = dlow + KD // 2
                tiles = []
                for name, src in (
                    ("ct", cos_ext[dlow]),
                    ("st", sin_ext[dlow]),
                    ("ch", cos_ext[dhigh]),
                    ("sh", sin_ext[dhigh]),
                ):
                    t = csp.tile(
                        [P, sl.stop - sl.start], bf16, tag="cs", name=f"cs_{name}"
                    )
                    nc.scalar.dma_start(out=t, in_=src[:, sl])
                    tiles.append(t)
                return tiles

            def project_rope(w_ext, wpool, out_ap, post=None):
                """dlow-outer order (used for Q)."""
                for dlow in range(KD // 2):
                    cs_full = cs_load(dlow, slice(0, NOWN))
                    for nb in range(NB):
                        sl = slice(nb * FB, (nb + 1) * FB)
                        cs_tiles = [t[:, sl] for t in cs_full]
                        rope_pair(wpool, w_ext, dlow, nb, cs_tiles, out_ap, post)

            # ---- K projection + rope -> kt_local -> split AllGathers ----
            def k_out(dc, nb):
                t = strm.tile([P, FB], bf16, tag="ro", name=f"kt_{dc}_{nb}")
                return t

            def k_post(dc, nb, t):
                for jj in range(FB // P):
                    nc.gpsimd.dma_start(
                        out=kt_local[nb * 4 + jj][:, dc, :],
                        in_=t[:, jj * P : (jj + 1) * P],
                    )

            def emit_kt_gather(half, out_t):
                nc.gpsimd.collective_compute(
                    "AllGather",
                    mybir.AluOpType.bypass,
                    replica_groups=PAIRS,
                    ins=[kt_local[half * 4 : (half + 1) * 4].opt()],
                    outs=[out_t.opt()],
                )

            # nb-outer so each half of kt_local completes early and its
            # gather pipelines with the rest of the projections
            with tc.tile_pool(name="wk_pool", bufs=2) as wkp:
                for nb in range(NB):
                    for dlow in range(KD // 2):
                        if nb == 0 and dlow == 1:
                            emit_wv_load()
                        cs_tiles = cs_load(dlow, slice(nb * FB, (nb + 1) * FB))
                        rope_pair(wkp, wk_ext, dlow, nb, cs_tiles, k_out, k_post)
                    emit_kt_gather(nb, kt_ga if nb == 0 else kt_gb)

            # ---- V projection (natural layout) -> v_local -> split gathers ----
            for ncc in range(NCJ // 2):
                for wb in range(D // FB):
                    ps_v = psum.tile([P, FB], f32, tag="ps")
                    for k in range(KD):
                        nc.tensor.matmul(
                            ps_v,
                            lhsT=x_sb[:, k, ncc * P : (ncc + 1) * P],
                            rhs=wv_sb[:, k, wb * FB : (wb + 1) * FB],
                            start=(k == 0),
                            stop=(k == KD - 1),
                        )
                    v_t = strm.tile([P, FB], bf16, tag="vo")
                    nc.vector.tensor_copy(v_t, ps_v)
                    nc.gpsimd.dma_start(
                        out=v_local[ncc][:, wb * FB : (wb + 1) * FB], in_=v_t
                    )
                if ncc == 3 or ncc == 7:
                    half = ncc // 4
                    nc.gpsimd.collective_compute(
                        "AllGather",
                        mybir.AluOpType.bypass,
                        replica_groups=PAIRS,
                        ins=[v_local[half * 4 : (half + 1) * 4].opt()],
                        outs=[(v_ga if half == 0 else v_gb).opt()],
                    )
            wv_pool.release()

            # ---- Q projection + rope (overlaps the collectives) ----
            def q_out(dc, nb):
                return qt_sb[:, dc, nb * FB : (nb + 1) * FB]

            with tc.tile_pool(name="wq_pool", bufs=2) as wqp:
                project_rope(wq_ext, wqp, q_out)
            x_pool.release()

            # ---- Attention ----
            with (
                tc.tile_pool(name="v2_pool", bufs=1) as v2p,
                tc.tile_pool(name="pt_pool", bufs=1) as ptp,
                tc.tile_pool(name="slab", bufs=6) as slab,
                tc.tile_pool(name="mskp", bufs=3) as mskp,
                tc.tile_pool(name="outp", bufs=4) as outp,
                tc.tile_pool(name="smallp", bufs=2) as smallp,
            ):
                v2_sb = v2p.tile([P, NCJ, D], bf16)
                for jc in range(NCJ):
                    nc.gpsimd.dma_start(out=v2_sb[:, jc, :], in_=v_g(jc))

                pt_sb = [
                    ptp.tile([P, NCJ, FB], bf16, name=f"pt_sb{ib}")
                    for ib in range(NB)
                ]

                # S^T = K @ Q^T; each kt slab is read once and used for both
                # i-blocks; mask loaded full-width per jc
                for jc in range(NCJ):
                    kt_slab = slab.tile([P, KD, P], bf16, tag="slab")
                    nc.sync.dma_start(out=kt_slab, in_=kt_g(jc))
                    msk = mskp.tile([P, NOWN], bf16, tag="m")
                    nc.scalar.dma_start(out=msk, in_=mask_ext[jc])
                    for ib in range(NB):
                        sl = slice(ib * FB, (ib + 1) * FB)
                        ps_s = psum.tile([P, FB], f32, tag="ps")
                        for k in range(KD):
                            nc.tensor.matmul(
                                ps_s,
                                lhsT=kt_slab[:, k, :],
                                rhs=qt_sb[:, k, sl],
                                start=(k == 0),
                                stop=(k == KD - 1),
                            )
                        tm = tmp.tile([P, FB], f32, tag="t")
                        nc.vector.tensor_mul(tm, ps_s, msk[:, sl])
                        nc.scalar.activation(
                            out=pt_sb[ib][:, jc, :],
                            in_=tm,
                            func=mybir.ActivationFunctionType.Exp,
                            scale=SCALE,
                        )

                # denominators + reciprocal broadcasts
                rbs = []
                for ib in range(NB):
                    ps_d = dnsum.tile([1, FB], f32, tag="dn")
                    for jc in range(NCJ):
                        nc.tensor.matmul(
                            ps_d,
                            lhsT=ones_col,
                            rhs=pt_sb[ib][:, jc, :],
                            start=(jc == 0),
                            stop=(jc == NCJ - 1),
                        )
                    recip = smallp.tile([1, FB], f32, tag="rc")
                    nc.vector.reciprocal(recip, ps_d)
                    ps_rb = dnsum.tile([P, FB], f32, tag="rb")
                    nc.tensor.matmul(
                        ps_rb, lhsT=ones_row, rhs=recip, start=True, stop=True
                    )
                    rb = smallp.tile([P, FB], f32, tag="rbs")
                    nc.vector.tensor_copy(rb, ps_rb)
                    rbs.append(rb)

                # O^T = V^T @ P^T, scaled by 1/denom
                for ib in range(NB):
                    rb = rbs[ib]
                    for dc in range(KD):
                        ps_o = psum.tile([P, FB], f32, tag="ps")
                        for jc in range(NCJ):
                            nc.tensor.matmul(
                                ps_o,
                                lhsT=v2_sb[:, jc, dc * P : (dc + 1) * P],
                                rhs=pt_sb[ib][:, jc, :],
                                start=(jc == 0),
                                stop=(jc == NCJ - 1),
                            )
                        o_st = outp.tile([P, FB], f32, tag="o")
                        nc.vector.tensor_mul(o_st, ps_o, rb)
                        nc.gpsimd.dma_start(
                            out=out_ext[
                                dc * P : (dc + 1) * P, ib * FB : (ib + 1) * FB
                            ],
                            in_=o_st,
                        )

    nc.compile()
    return nc


def _prep_inputs(x, cos, sin, Wq, Wk, Wv):
    """Host-side sharding/layout prep. Returns in_maps for 8 cores."""
    x = np.asarray(x, dtype=np.float32)
    cos = np.asarray(cos, dtype=np.float32)
    sin = np.asarray(sin, dtype=np.float32)

    def w_panels(w):
        # W.T [din, dout] -> [dc, p_din, k_din, c_dout] with d = k*128+p
        wt = np.ascontiguousarray(np.asarray(w, dtype=np.float32).T).astype(BF16)
        return np.ascontiguousarray(
            wt.reshape(KD, P, KD, P).transpose(2, 1, 0, 3)
        )

    wq_p = w_panels(Wq)
    wk_p = w_panels(Wk)
    # Wv.T [din, dout] -> [p, k, dout]
    wv_p = np.ascontiguousarray(
        np.asarray(Wv, dtype=np.float32).T.astype(BF16).reshape(KD, P, D).transpose(1, 0, 2)
    )

    in_maps = []
    for c in range(N_CORES):
        b, h = divmod(c, 2)
        rows = slice(h * NOWN, (h + 1) * NOWN)
        xt = np.ascontiguousarray(
            x[b, rows, :].T.astype(BF16).reshape(KD, P, NOWN).transpose(1, 0, 2)
        )
        cos_t = np.ascontiguousarray(cos[rows].T.astype(BF16).reshape(KD, P, NOWN))
        sin_t = np.ascontiguousarray(sin[rows].T.astype(BF16).reshape(KD, P, NOWN))
        j = np.arange(S, dtype=np.int64)[:, None]
        i = (np.arange(NOWN, dtype=np.int64) + h * NOWN)[None, :]
        mask_t = (j <= i).astype(BF16).reshape(NCJ, P, NOWN)
        in_maps.append(
            {
                "x_t": xt,
                "wq": wq_p,
                "wk": wk_p,
                "wv": wv_p,
                "cos_t": cos_t,
                "sin_t": sin_t,
                "mask_t": mask_t,
            }
        )
    return in_maps


def _run(in_maps, trace=False, tmpdir=None):
    from concourse.bass_utils import run_bass_kernel_spmd

    if "nc" not in _CACHE:
        _CACHE["nc"] = _build()
    nc = _CACHE["nc"]
    return run_bass_kernel_spmd(
        nc, in_maps, list(range(N_CORES)), trace=trace, tmpdir=tmpdir
    )


def kernel(x, cos, sin, Wq, Wk, Wv):
    in_maps = _prep_inputs(x, cos, sin, Wq, Wk, Wv)
    res = _run(in_maps, trace=False)
    out = np.empty((B, S, D), dtype=np.float32)
    for c in range(N_CORES):
        b, h = divmod(c, 2)
        out[b, h * NOWN : (h + 1) * NOWN, :] = res.results[c]["out"].T
    return out


# revision 37
# speedup vs baseline: 1.0493x; 1.0109x over previous
"""Trainium2 Bass kernel: single-head attention with RoPE and the reference's
multiplicative causal mask (masked logits stay 0 -> exp(0)=1, dense attention).

Sharding: 8 cores = 4 batches x 2 sequence-halves. Each core projects Q/K/V
for its 1024 rows (bf16 matmuls, fp32 PSUM), applies RoPE on-chip, all-gathers
roped K and V within the 2-core pair, then computes attention for its rows.
Output is O^T per core; the host transposes and reassembles.
"""

import sys

for _p in ("/opt/trn_rl_repo", "/root/.axon_site/_ro/trn_rl_repo"):
    if _p not in sys.path:
        sys.path.append(_p)

import math

import ml_dtypes
import numpy as np

BF16 = ml_dtypes.bfloat16

B, S, D = 4, 2048, 2048
NOWN = 1024  # query rows per core
P = 128  # partitions
KD = D // P  # 16 feature chunks
NCJ = S // P  # 16 key chunks
N_CORES = 8
PAIRS = [[0, 1], [2, 3], [4, 5], [6, 7]]
FB = 512  # matmul moving free-dim block
NB = NOWN // FB  # 2 blocks of own rows
SCALE = 1.0 / math.sqrt(S)  # reference scales by sqrt(seq_len), not sqrt(D)

_CACHE = {}


def _build():
    import concourse.bass as bass  # noqa: F401
    import concourse.tile as tile
    from concourse import bacc, mybir

    f32 = mybir.dt.float32
    bf16 = mybir.dt.bfloat16

    nc = bacc.Bacc(
        "TRN2", target_bir_lowering=False, debug=False, num_devices=N_CORES
    )

    x_ext = nc.dram_tensor("x_t", [P, KD, NOWN], bf16, kind="ExternalInput").ap()
    wq_ext = nc.dram_tensor("wq", [KD, P, KD, P], bf16, kind="ExternalInput").ap()
    wk_ext = nc.dram_tensor("wk", [KD, P, KD, P], bf16, kind="ExternalInput").ap()
    wv_ext = nc.dram_tensor("wv", [P, KD, D], bf16, kind="ExternalInput").ap()
    cos_ext = nc.dram_tensor("cos_t", [KD, P, NOWN], bf16, kind="ExternalInput").ap()
    sin_ext = nc.dram_tensor("sin_t", [KD, P, NOWN], bf16, kind="ExternalInput").ap()
    mask_ext = nc.dram_tensor("mask_t", [NCJ, P, NOWN], bf16, kind="ExternalInput").ap()
    out_ext = nc.dram_tensor("out", [D, NOWN], f32, kind="ExternalOutput").ap()

    with tile.TileContext(nc) as tc:
        with (
            tc.tile_pool(name="dram", bufs=1, space="DRAM") as dram,
            tc.tile_pool(name="psum", bufs=6, space="PSUM") as psum,
            tc.tile_pool(name="dnsum", bufs=1, space="PSUM") as dnsum,
            tc.tile_pool(name="persist", bufs=1) as persist,
            tc.tile_pool(name="tmp", bufs=4) as tmp,
            tc.tile_pool(name="csp", bufs=4) as csp,
            tc.tile_pool(name="strm", bufs=8) as strm,
        ):
            kt_local = dram.tile([NCJ // 2, P, KD, P], bf16)
            v_local = dram.tile([NCJ // 2, P, D], bf16)
            # gathered tensors, split in halves so each 2MB gather can launch
            # as soon as its half is produced (pipelines with compute)
            kt_ga = dram.tile([2, 4, P, KD, P], bf16)
            kt_gb = dram.tile([2, 4, P, KD, P], bf16)
            v_ga = dram.tile([2, 4, P, D], bf16)
            v_gb = dram.tile([2, 4, P, D], bf16)

            def kt_g(jc):
                h2, jcl = jc // 8, jc % 8
                return (kt_ga if jcl < 4 else kt_gb)[h2, jcl % 4]

            def v_g(jc):
                h2, jcl = jc // 8, jc % 8
                return (v_ga if jcl < 4 else v_gb)[h2, jcl % 4]

            ones_col = persist.tile([P, 1], bf16)
            nc.vector.memset(ones_col, 1.0)
            ones_row = persist.tile([1, P], f32)
            nc.vector.memset(ones_row, 1.0)

            # warm up the PE (HAM clock gate) while the input DMAs land:
            # dummy matmuls on a const tile, result never read
            warm_in = persist.tile([P, FB], bf16)
            nc.vector.memset(warm_in, 0.0)
            ps_warm = psum.tile([P, FB], f32, tag="ps")
            for w in range(40):
                nc.tensor.matmul(
                    ps_warm,
                    lhsT=warm_in[:, 0:P],
                    rhs=warm_in,
                    start=(w == 0),
                    stop=(w == 39),
                )

            x_pool = tc.alloc_tile_pool(name="x_pool", bufs=1)
            x_sb = x_pool.tile([P, KD, NOWN], bf16)
            # split the load across engines/queues so the K matmuls start early
            x_dma_engines = [nc.sync, nc.scalar, nc.gpsimd, nc.sync]
            for kg in range(8):
                x_dma_engines[kg % 4].dma_start(
                    out=x_sb[:, kg * 2 : (kg + 1) * 2, :],
                    in_=x_ext[:, kg * 2 : (kg + 1) * 2, :],
                )

            # wv lives in its own pool; the load is emitted mid-K-phase so it
            # overlaps K compute without stealing startup DMA bandwidth from x
            wv_pool = tc.alloc_tile_pool(name="wv_pool", bufs=1)
            wv_sb = wv_pool.tile([P, KD, D], bf16)

            def emit_wv_load():
                for kg in range(4):
                    eng = nc.scalar if kg % 2 == 0 else nc.gpsimd
                    eng.dma_start(
                        out=wv_sb[:, kg * 4 : (kg + 1) * 4, :],
                        in_=wv_ext[:, kg * 4 : (kg + 1) * 4, :],
                    )

            qt_sb = persist.tile([P, KD, NOWN], bf16)

            def load_panels(wpool, w_ext, dlow, nb):
                dhigh = dlow + KD // 2
                w_lo = wpool.tile([P, KD, P], bf16, tag="wp", name=f"wlo{dlow}{nb}")
                nc.sync.dma_start(out=w_lo, in_=w_ext[dlow])
                w_hi = wpool.tile([P, KD, P], bf16, tag="wp", name=f"whi{dlow}{nb}")
                nc.sync.dma_start(out=w_hi, in_=w_ext[dhigh])
                return w_lo, w_hi

            def rope_pair(panels, dlow, nb, cs_tiles, out_ap, post):
                """One (dlow, nb) unit: two projections + rope."""
                dhigh = dlow + KD // 2
                sl = slice(nb * FB, (nb + 1) * FB)
                cos_t, sin_t, cos_h, sin_h = cs_tiles
                w_lo, w_hi = panels
                ps_lo = psum.tile([P, FB], f32, tag="ps", name=f"plo{dlow}{nb}")
                for k in range(KD):
                    nc.tensor.matmul(
                        ps_lo,
                        lhsT=w_lo[:, k, :],
                        rhs=x_sb[:, k, sl],
                        start=(k == 0),
                        stop=(k == KD - 1),
                    )
                ps_hi = psum.tile([P, FB], f32, tag="ps", name=f"phi{dlow}{nb}")
                for k in range(KD):
                    nc.tensor.matmul(
                        ps_hi,
                        lhsT=w_hi[:, k, :],
                        rhs=x_sb[:, k, sl],
                        start=(k == 0),
                        stop=(k == KD - 1),
                    )
                # rope low half: out = lo*cos_l - hi*sin_l
                t1 = tmp.tile([P, FB], f32, tag="t", name=f"t1{dlow}{nb}")
                nc.vector.tensor_mul(t1, ps_lo, cos_t)
                t2 = tmp.tile([P, FB], f32, tag="t", name=f"t2{dlow}{nb}")
                nc.vector.tensor_mul(t2, ps_hi, sin_t)
                o_lo = out_ap(dlow, nb)
                nc.vector.tensor_sub(o_lo, t1, t2)
                if post is not None:
                    post(dlow, nb, o_lo)
                # rope high half: out = hi*cos_h + lo*sin_h
                t3 = tmp.tile([P, FB], f32, tag="t", name=f"t3{dlow}{nb}")
                nc.vector.tensor_mul(t3, ps_hi, cos_h)
                t4 = tmp.tile([P, FB], f32, tag="t", name=f"t4{dlow}{nb}")
                nc.vector.tensor_mul(t4, ps_lo, sin_h)
                o_hi = out_ap(dhigh, nb)
                nc.vector.tensor_add(o_hi, t3, t4)
                if post is not None:
                    post(dhigh, nb, o_hi)

            def cs_load(dlow, sl):
                dhigh = dlow + KD // 2
                tiles = []
                for name, src in (
                    ("ct", cos_ext[dlow]),
                    ("st", sin_ext[dlow]),
                    ("ch", cos_ext[dhigh]),
                    ("sh", sin_ext[dhigh]),
                ):
                    t = csp.tile(
                        [P, sl.stop - sl.start], bf16, tag="cs", name=f"cs_{name}"
                    )
                    nc.scalar.dma_start(out=t, in_=src[:, sl])
                    tiles.append(t)
                return tiles

            def project_rope(w_ext, wpool, out_ap, post=None):
                """dlow-outer order (used for Q); panels loaded once per dlow."""
                for dlow in range(KD // 2):
                    cs_full = cs_load(dlow, slice(0, NOWN))
                    panels = load_panels(wpool, w_ext, dlow, 0)
                    for nb in range(NB):
                        sl = slice(nb * FB, (nb + 1) * FB)
                        cs_tiles = [t[:, sl] for t in cs_full]
                        rope_pair(panels, dlow, nb, cs_tiles, out_ap, post)

            # ---- K projection + rope -> kt_local -> split AllGathers ----
            def k_out(dc, nb):
                t = strm.tile([P, FB], bf16, tag="ro", name=f"kt_{dc}_{nb}")
                return t

            def k_post(dc, nb, t):
                for jj in range(FB // P):
                    nc.gpsimd.dma_start(
                        out=kt_local[nb * 4 + jj][:, dc, :],
                        in_=t[:, jj * P : (jj + 1) * P],
                    )

            def emit_kt_gather(half, out_t):
                nc.gpsimd.collective_compute(
                    "AllGather",
                    mybir.AluOpType.bypass,
                    replica_groups=PAIRS,
                    ins=[kt_local[half * 4 : (half + 1) * 4].opt()],
                    outs=[out_t.opt()],
                )

            # nb-outer so each half of kt_local completes early and its
            # gather pipelines with the rest of the projections
            with tc.tile_pool(name="wk_pool", bufs=2) as wkp:
                for nb in range(NB):
                    for dlow in range(KD // 2):
                        if nb == 0 and dlow == 1:
                            emit_wv_load()
                        cs_tiles = cs_load(dlow, slice(nb * FB, (nb + 1) * FB))
                        panels = load_panels(wkp, wk_ext, dlow, nb)
                        rope_pair(panels, dlow, nb, cs_tiles, k_out, k_post)
                    emit_kt_gather(nb, kt_ga if nb == 0 else kt_gb)

            # ---- V projection (natural layout) -> v_local -> split gathers ----
            for ncc in range(NCJ // 2):
                for wb in range(D // FB):
                    ps_v = psum.tile([P, FB], f32, tag="ps")
                    for k in range(KD):
                        nc.tensor.matmul(
                            ps_v,
                            lhsT=x_sb[:, k, ncc * P : (ncc + 1) * P],
                            rhs=wv_sb[:, k, wb * FB : (wb + 1) * FB],
                            start=(k == 0),
                            stop=(k == KD - 1),
                        )
                    v_t = strm.tile([P, FB], bf16, tag="vo")
                    nc.vector.tensor_copy(v_t, ps_v)
                    nc.gpsimd.dma_start(
                        out=v_local[ncc][:, wb * FB : (wb + 1) * FB], in_=v_t
                    )
                if ncc == 3 or ncc == 7:
                    half = ncc // 4
                    nc.gpsimd.collective_compute(
                        "AllGather",
                        mybir.AluOpType.bypass,
                        replica_groups=PAIRS,
                        ins=[v_local[half * 4 : (half + 1) * 4].opt()],
                        outs=[(v_ga if half == 0 else v_gb).opt()],
                    )
            wv_pool.release()

            # ---- Q projection + rope (overlaps the collectives) ----
            def q_out(dc, nb):
                return qt_sb[:, dc, nb * FB : (nb + 1) * FB]

            with tc.tile_pool(name="wq_pool", bufs=2) as wqp:
                project_rope(wq_ext, wqp, q_out)
            x_pool.release()

            # ---- Attention ----
            with (
                tc.tile_pool(name="v2_pool", bufs=1) as v2p,
                tc.tile_pool(name="pt_pool", bufs=1) as ptp,
                tc.tile_pool(name="slab", bufs=6) as slab,
                tc.tile_pool(name="mskp", bufs=3) as mskp,
                tc.tile_pool(name="outp", bufs=4) as outp,
                tc.tile_pool(name="smallp", bufs=2) as smallp,
            ):
                v2_sb = v2p.tile([P, NCJ, D], bf16)
                for jc in range(NCJ):
                    nc.gpsimd.dma_start(out=v2_sb[:, jc, :], in_=v_g(jc))

                pt_sb = [
                    ptp.tile([P, NCJ, FB], bf16, name=f"pt_sb{ib}")
                    for ib in range(NB)
                ]

                # S^T = K @ Q^T; each kt slab is read once and used for both
                # i-blocks; mask loaded full-width per jc
                for jc in range(NCJ):
                    kt_slab = slab.tile([P, KD, P], bf16, tag="slab")
                    nc.sync.dma_start(out=kt_slab, in_=kt_g(jc))
                    msk = mskp.tile([P, NOWN], bf16, tag="m")
                    nc.scalar.dma_start(out=msk, in_=mask_ext[jc])
                    for ib in range(NB):
                        sl = slice(ib * FB, (ib + 1) * FB)
                        ps_s = psum.tile([P, FB], f32, tag="ps")
                        for k in range(KD):
                            nc.tensor.matmul(
                                ps_s,
                                lhsT=kt_slab[:, k, :],
                                rhs=qt_sb[:, k, sl],
                                start=(k == 0),
                                stop=(k == KD - 1),
                            )
                        tm = tmp.tile([P, FB], f32, tag="t")
                        nc.vector.tensor_mul(tm, ps_s, msk[:, sl])
                        nc.scalar.activation(
                            out=pt_sb[ib][:, jc, :],
                            in_=tm,
                            func=mybir.ActivationFunctionType.Exp,
                            scale=SCALE,
                        )

                # denominators + reciprocal broadcasts
                rbs = []
                for ib in range(NB):
                    ps_d = dnsum.tile([1, FB], f32, tag="dn")
                    for jc in range(NCJ):
                        nc.tensor.matmul(
                            ps_d,
                            lhsT=ones_col,
                            rhs=pt_sb[ib][:, jc, :],
                            start=(jc == 0),
                            stop=(jc == NCJ - 1),
                        )
                    recip = smallp.tile([1, FB], f32, tag="rc")
                    nc.vector.reciprocal(recip, ps_d)
                    ps_rb = dnsum.tile([P, FB], f32, tag="rb")
                    nc.tensor.matmul(
                        ps_rb, lhsT=ones_row, rhs=recip, start=True, stop=True
                    )
                    rb = smallp.tile([P, FB], f32, tag="rbs")
                    nc.vector.tensor_copy(rb, ps_rb)
                    rbs.append(rb)

                # O^T = V^T @ P^T, scaled by 1/denom
                for ib in range(NB):
                    rb = rbs[ib]
                    for dc in range(KD):
                        ps_o = psum.tile([P, FB], f32, tag="ps")
                        for jc in range(NCJ):
                            nc.tensor.matmul(
                                ps_o,
                                lhsT=v2_sb[:, jc, dc * P : (dc + 1) * P],
                                rhs=pt_sb[ib][:, jc, :],
                                start=(jc == 0),
                                stop=(jc == NCJ - 1),
                            )
                        o_st = outp.tile([P, FB], f32, tag="o")
                        nc.vector.tensor_mul(o_st, ps_o, rb)
                        nc.gpsimd.dma_start(
                            out=out_ext[
                                dc * P : (dc + 1) * P, ib * FB : (ib + 1) * FB
                            ],
                            in_=o_st,
                        )

    nc.compile()
    return nc


def _prep_inputs(x, cos, sin, Wq, Wk, Wv):
    """Host-side sharding/layout prep. Returns in_maps for 8 cores."""
    x = np.asarray(x, dtype=np.float32)
    cos = np.asarray(cos, dtype=np.float32)
    sin = np.asarray(sin, dtype=np.float32)

    def w_panels(w):
        # W.T [din, dout] -> [dc, p_din, k_din, c_dout] with d = k*128+p
        wt = np.ascontiguousarray(np.asarray(w, dtype=np.float32).T).astype(BF16)
        return np.ascontiguousarray(
            wt.reshape(KD, P, KD, P).transpose(2, 1, 0, 3)
        )

    wq_p = w_panels(Wq)
    wk_p = w_panels(Wk)
    # Wv.T [din, dout] -> [p, k, dout]
    wv_p = np.ascontiguousarray(
        np.asarray(Wv, dtype=np.float32).T.astype(BF16).reshape(KD, P, D).transpose(1, 0, 2)
    )

    in_maps = []
    for c in range(N_CORES):
        b, h = divmod(c, 2)
        rows = slice(h * NOWN, (h + 1) * NOWN)
        xt = np.ascontiguousarray(
            x[b, rows, :].T.astype(BF16).reshape(KD, P, NOWN).transpose(1, 0, 2)
        )
        cos_t = np.ascontiguousarray(cos[rows].T.astype(BF16).reshape(KD, P, NOWN))
        sin_t = np.ascontiguousarray(sin[rows].T.astype(BF16).reshape(KD, P, NOWN))
        j = np.arange(S, dtype=np.int64)[:, None]
        i = (np.arange(NOWN, dtype=np.int64) + h * NOWN)[None, :]
        mask_t = (j <= i).astype(BF16).reshape(NCJ, P, NOWN)
        in_maps.append(
            {
                "x_t": xt,
                "wq": wq_p,
                "wk": wk_p,
                "wv": wv_p,
                "cos_t": cos_t,
                "sin_t": sin_t,
                "mask_t": mask_t,
            }
        )
    return in_maps


def _run(in_maps, trace=False, tmpdir=None):
    from concourse.bass_utils import run_bass_kernel_spmd

    if "nc" not in _CACHE:
        _CACHE["nc"] = _build()
    nc = _CACHE["nc"]
    return run_bass_kernel_spmd(
        nc, in_maps, list(range(N_CORES)), trace=trace, tmpdir=tmpdir
    )


def kernel(x, cos, sin, Wq, Wk, Wv):
    in_maps = _prep_inputs(x, cos, sin, Wq, Wk, Wv)
    res = _run(in_maps, trace=False)
    out = np.empty((B, S, D), dtype=np.float32)
    for c in range(N_CORES):
        b, h = divmod(c, 2)
        out[b, h * NOWN : (h + 1) * NOWN, :] = res.results[c]["out"].T
    return out


# revision 40
# speedup vs baseline: 1.1262x; 1.0733x over previous
"""Trainium2 Bass kernel: single-head attention with RoPE and the reference's
multiplicative causal mask (masked logits stay 0 -> exp(0)=1, dense attention).

Sharding: 8 cores = 4 batches x 2 sequence-halves. Each core projects Q/K/V
for its 1024 rows (bf16 matmuls, fp32 PSUM), applies RoPE on-chip, all-gathers
roped K and V within the 2-core pair, then computes attention for its rows.
Output is O^T per core; the host transposes and reassembles.
"""

import sys

for _p in ("/opt/trn_rl_repo", "/root/.axon_site/_ro/trn_rl_repo"):
    if _p not in sys.path:
        sys.path.append(_p)

import math

import ml_dtypes
import numpy as np

BF16 = ml_dtypes.bfloat16

B, S, D = 4, 2048, 2048
NOWN = 1024  # query rows per core
P = 128  # partitions
KD = D // P  # 16 feature chunks
NCJ = S // P  # 16 key chunks
N_CORES = 8
PAIRS = [[0, 1], [2, 3], [4, 5], [6, 7]]
FB = 512  # matmul moving free-dim block
NB = NOWN // FB  # 2 blocks of own rows
SCALE = 1.0 / math.sqrt(S)  # reference scales by sqrt(seq_len), not sqrt(D)

_CACHE = {}


def _build():
    import concourse.bass as bass  # noqa: F401
    import concourse.tile as tile
    from concourse import bacc, mybir

    f32 = mybir.dt.float32
    bf16 = mybir.dt.bfloat16

    nc = bacc.Bacc(
        "TRN2", target_bir_lowering=False, debug=False, num_devices=N_CORES
    )

    x_ext = nc.dram_tensor("x_t", [P, KD, NOWN], bf16, kind="ExternalInput").ap()
    wq_ext = nc.dram_tensor("wq", [KD, P, KD, P], bf16, kind="ExternalInput").ap()
    wk_ext = nc.dram_tensor("wk", [KD, P, KD, P], bf16, kind="ExternalInput").ap()
    wv_ext = nc.dram_tensor("wv", [P, KD, D], bf16, kind="ExternalInput").ap()
    cos_ext = nc.dram_tensor("cos_t", [KD, P, NOWN], bf16, kind="ExternalInput").ap()
    sin_ext = nc.dram_tensor("sin_t", [KD, P, NOWN], bf16, kind="ExternalInput").ap()
    mask_ext = nc.dram_tensor("mask_t", [NCJ, P, NOWN], bf16, kind="ExternalInput").ap()
    out_ext = nc.dram_tensor("out", [D, NOWN], f32, kind="ExternalOutput").ap()

    with tile.TileContext(nc) as tc:
        with (
            tc.tile_pool(name="dram", bufs=1, space="DRAM") as dram,
            tc.tile_pool(name="psum", bufs=6, space="PSUM") as psum,
            tc.tile_pool(name="dnsum", bufs=1, space="PSUM") as dnsum,
            tc.tile_pool(name="persist", bufs=1) as persist,
            tc.tile_pool(name="tmp", bufs=4) as tmp,
            tc.tile_pool(name="csp", bufs=4) as csp,
            tc.tile_pool(name="strm", bufs=8) as strm,
        ):
            kt_local = dram.tile([NCJ // 2, P, KD, P], bf16)
            v_local = dram.tile([NCJ // 2, P, D], bf16)
            # gathered tensors, split in halves so each 2MB gather can launch
            # as soon as its half is produced (pipelines with compute)
            kt_ga = dram.tile([2, 4, P, KD, P], bf16)
            kt_gb = dram.tile([2, 4, P, KD, P], bf16)
            v_ga = dram.tile([2, 4, P, D], bf16)
            v_gb = dram.tile([2, 4, P, D], bf16)

            def kt_g(jc):
                h2, jcl = jc // 8, jc % 8
                return (kt_ga if jcl < 4 else kt_gb)[h2, jcl % 4]

            def v_g(jc):
                h2, jcl = jc // 8, jc % 8
                return (v_ga if jcl < 4 else v_gb)[h2, jcl % 4]

            ones_col = persist.tile([P, 1], bf16)
            nc.vector.memset(ones_col, 1.0)
            ones_row = persist.tile([1, P], f32)
            nc.vector.memset(ones_row, 1.0)



            x_pool = tc.alloc_tile_pool(name="x_pool", bufs=1)
            x_sb = x_pool.tile([P, KD, NOWN], bf16)
            # split the load across engines/queues so the K matmuls start early
            x_dma_engines = [nc.sync, nc.scalar, nc.gpsimd, nc.sync]
            for kg in range(8):
                x_dma_engines[kg % 4].dma_start(
                    out=x_sb[:, kg * 2 : (kg + 1) * 2, :],
                    in_=x_ext[:, kg * 2 : (kg + 1) * 2, :],
                )

            # wv lives in its own pool; the load is emitted mid-K-phase so it
            # overlaps K compute without stealing startup DMA bandwidth from x
            wv_pool = tc.alloc_tile_pool(name="wv_pool", bufs=1)
            wv_sb = wv_pool.tile([P, KD, D], bf16)

            def emit_wv_load():
                for kg in range(4):
                    eng = nc.scalar if kg % 2 == 0 else nc.gpsimd
                    eng.dma_start(
                        out=wv_sb[:, kg * 4 : (kg + 1) * 4, :],
                        in_=wv_ext[:, kg * 4 : (kg + 1) * 4, :],
                    )

            qt_sb = persist.tile([P, KD, NOWN], bf16)

            def load_panels(wpool, w_ext, dlow, nb):
                dhigh = dlow + KD // 2
                w_lo = wpool.tile([P, KD, P], bf16, tag="wp", name=f"wlo{dlow}{nb}")
                nc.sync.dma_start(out=w_lo, in_=w_ext[dlow])
                w_hi = wpool.tile([P, KD, P], bf16, tag="wp", name=f"whi{dlow}{nb}")
                nc.sync.dma_start(out=w_hi, in_=w_ext[dhigh])
                return w_lo, w_hi

            def rope_pair(panels, dlow, nb, cs_tiles, out_ap, post):
                """One (dlow, nb) unit: two projections + rope."""
                dhigh = dlow + KD // 2
                sl = slice(nb * FB, (nb + 1) * FB)
                cos_t, sin_t, cos_h, sin_h = cs_tiles
                w_lo, w_hi = panels
                ps_lo = psum.tile([P, FB], f32, tag="ps", name=f"plo{dlow}{nb}")
                for k in range(KD):
                    nc.tensor.matmul(
                        ps_lo,
                        lhsT=w_lo[:, k, :],
                        rhs=x_sb[:, k, sl],
                        start=(k == 0),
                        stop=(k == KD - 1),
                    )
                ps_hi = psum.tile([P, FB], f32, tag="ps", name=f"phi{dlow}{nb}")
                for k in range(KD):
                    nc.tensor.matmul(
                        ps_hi,
                        lhsT=w_hi[:, k, :],
                        rhs=x_sb[:, k, sl],
                        start=(k == 0),
                        stop=(k == KD - 1),
                    )
                # rope low half: out = lo*cos_l - hi*sin_l
                t1 = tmp.tile([P, FB], f32, tag="t", name=f"t1{dlow}{nb}")
                nc.vector.tensor_mul(t1, ps_lo, cos_t)
                t2 = tmp.tile([P, FB], f32, tag="t", name=f"t2{dlow}{nb}")
                nc.vector.tensor_mul(t2, ps_hi, sin_t)
                o_lo = out_ap(dlow, nb)
                nc.vector.tensor_sub(o_lo, t1, t2)
                if post is not None:
                    post(dlow, nb, o_lo)
                # rope high half: out = hi*cos_h + lo*sin_h
                t3 = tmp.tile([P, FB], f32, tag="t", name=f"t3{dlow}{nb}")
                nc.vector.tensor_mul(t3, ps_hi, cos_h)
                t4 = tmp.tile([P, FB], f32, tag="t", name=f"t4{dlow}{nb}")
                nc.vector.tensor_mul(t4, ps_lo, sin_h)
                o_hi = out_ap(dhigh, nb)
                nc.vector.tensor_add(o_hi, t3, t4)
                if post is not None:
                    post(dhigh, nb, o_hi)

            def cs_load(dlow, sl):
                dhigh = dlow + KD // 2
                tiles = []
                for name, src in (
                    ("ct", cos_ext[dlow]),
                    ("st", sin_ext[dlow]),
                    ("ch", cos_ext[dhigh]),
                    ("sh", sin_ext[dhigh]),
                ):
                    t = csp.tile(
                        [P, sl.stop - sl.start], bf16, tag="cs", name=f"cs_{name}"
                    )
                    nc.scalar.dma_start(out=t, in_=src[:, sl])
                    tiles.append(t)
                return tiles

            def project_rope(w_ext, wpool, out_ap, post=None):
                """dlow-outer order (used for Q); panels loaded once per dlow."""
                for dlow in range(KD // 2):
                    cs_full = cs_load(dlow, slice(0, NOWN))
                    panels = load_panels(wpool, w_ext, dlow, 0)
                    for nb in range(NB):
                        sl = slice(nb * FB, (nb + 1) * FB)
                        cs_tiles = [t[:, sl] for t in cs_full]
                        rope_pair(panels, dlow, nb, cs_tiles, out_ap, post)

            # ---- K projection + rope -> kt_local -> split AllGathers ----
            def k_out(dc, nb):
                t = strm.tile([P, FB], bf16, tag="ro", name=f"kt_{dc}_{nb}")
                return t

            def k_post(dc, nb, t):
                for jj in range(FB // P):
                    nc.gpsimd.dma_start(
                        out=kt_local[nb * 4 + jj][:, dc, :],
                        in_=t[:, jj * P : (jj + 1) * P],
                    )

            def emit_kt_gather(half, out_t):
                nc.gpsimd.collective_compute(
                    "AllGather",
                    mybir.AluOpType.bypass,
                    replica_groups=PAIRS,
                    ins=[kt_local[half * 4 : (half + 1) * 4].opt()],
                    outs=[out_t.opt()],
                )

            # nb-outer so each half of kt_local completes early and its
            # gather pipelines with the rest of the projections
            with tc.tile_pool(name="wk_pool", bufs=6) as wkp:
                for nb in range(NB):
                    for dlow in range(KD // 2):
                        if nb == 0 and dlow == 1:
                            emit_wv_load()
                        cs_tiles = cs_load(dlow, slice(nb * FB, (nb + 1) * FB))
                        panels = load_panels(wkp, wk_ext, dlow, nb)
                        rope_pair(panels, dlow, nb, cs_tiles, k_out, k_post)
                    emit_kt_gather(nb, kt_ga if nb == 0 else kt_gb)

            # ---- V projection (natural layout) -> v_local -> split gathers ----
            for ncc in range(NCJ // 2):
                for wb in range(D // FB):
                    ps_v = psum.tile([P, FB], f32, tag="ps")
                    for k in range(KD):
                        nc.tensor.matmul(
                            ps_v,
                            lhsT=x_sb[:, k, ncc * P : (ncc + 1) * P],
                            rhs=wv_sb[:, k, wb * FB : (wb + 1) * FB],
                            start=(k == 0),
                            stop=(k == KD - 1),
                        )
                    v_t = strm.tile([P, FB], bf16, tag="vo")
                    nc.vector.tensor_copy(v_t, ps_v)
                    nc.gpsimd.dma_start(
                        out=v_local[ncc][:, wb * FB : (wb + 1) * FB], in_=v_t
                    )
                if ncc == 3 or ncc == 7:
                    half = ncc // 4
                    nc.gpsimd.collective_compute(
                        "AllGather",
                        mybir.AluOpType.bypass,
                        replica_groups=PAIRS,
                        ins=[v_local[half * 4 : (half + 1) * 4].opt()],
                        outs=[(v_ga if half == 0 else v_gb).opt()],
                    )
            wv_pool.release()

            # ---- Q projection + rope (overlaps the collectives) ----
            def q_out(dc, nb):
                return qt_sb[:, dc, nb * FB : (nb + 1) * FB]

            with tc.tile_pool(name="wq_pool", bufs=6) as wqp:
                project_rope(wq_ext, wqp, q_out)
            x_pool.release()

            # ---- Attention ----
            with (
                tc.tile_pool(name="v2_pool", bufs=1) as v2p,
                tc.tile_pool(name="pt_pool", bufs=1) as ptp,
                tc.tile_pool(name="slab", bufs=6) as slab,
                tc.tile_pool(name="mskp", bufs=3) as mskp,
                tc.tile_pool(name="outp", bufs=4) as outp,
                tc.tile_pool(name="smallp", bufs=2) as smallp,
            ):
                v2_sb = v2p.tile([P, NCJ, D], bf16)
                for jc in range(NCJ):
                    nc.gpsimd.dma_start(out=v2_sb[:, jc, :], in_=v_g(jc))

                pt_sb = [
                    ptp.tile([P, NCJ, FB], bf16, name=f"pt_sb{ib}")
                    for ib in range(NB)
                ]

                # S^T = K @ Q^T; each kt slab is read once and used for both
                # i-blocks; mask loaded full-width per jc
                for jc in range(NCJ):
                    kt_slab = slab.tile([P, KD, P], bf16, tag="slab")
                    nc.sync.dma_start(out=kt_slab, in_=kt_g(jc))
                    msk = mskp.tile([P, NOWN], bf16, tag="m")
                    nc.scalar.dma_start(out=msk, in_=mask_ext[jc])
                    for ib in range(NB):
                        sl = slice(ib * FB, (ib + 1) * FB)
                        ps_s = psum.tile([P, FB], f32, tag="ps")
                        for k in range(KD):
                            nc.tensor.matmul(
                                ps_s,
                                lhsT=kt_slab[:, k, :],
                                rhs=qt_sb[:, k, sl],
                                start=(k == 0),
                                stop=(k == KD - 1),
                            )
                        tm = tmp.tile([P, FB], f32, tag="t")
                        nc.vector.tensor_mul(tm, ps_s, msk[:, sl])
                        nc.scalar.activation(
                            out=pt_sb[ib][:, jc, :],
                            in_=tm,
                            func=mybir.ActivationFunctionType.Exp,
                            scale=SCALE,
                        )

                # denominators + reciprocal broadcasts
                rbs = []
                for ib in range(NB):
                    ps_d = dnsum.tile([1, FB], f32, tag="dn")
                    for jc in range(NCJ):
                        nc.tensor.matmul(
                            ps_d,
                            lhsT=ones_col,
                            rhs=pt_sb[ib][:, jc, :],
                            start=(jc == 0),
                            stop=(jc == NCJ - 1),
                        )
                    recip = smallp.tile([1, FB], f32, tag="rc")
                    nc.vector.reciprocal(recip, ps_d)
                    ps_rb = dnsum.tile([P, FB], f32, tag="rb")
                    nc.tensor.matmul(
                        ps_rb, lhsT=ones_row, rhs=recip, start=True, stop=True
                    )
                    rb = smallp.tile([P, FB], f32, tag="rbs")
                    nc.vector.tensor_copy(rb, ps_rb)
                    rbs.append(rb)

                # O^T = V^T @ P^T, scaled by 1/denom
                for ib in range(NB):
                    rb = rbs[ib]
                    for dc in range(KD):
                        ps_o = psum.tile([P, FB], f32, tag="ps")
                        for jc in range(NCJ):
                            nc.tensor.matmul(
                                ps_o,
                                lhsT=v2_sb[:, jc, dc * P : (dc + 1) * P],
                                rhs=pt_sb[ib][:, jc, :],
                                start=(jc == 0),
                                stop=(jc == NCJ - 1),
                            )
                        o_st = outp.tile([P, FB], f32, tag="o")
                        nc.vector.tensor_mul(o_st, ps_o, rb)
                        nc.gpsimd.dma_start(
                            out=out_ext[
                                dc * P : (dc + 1) * P, ib * FB : (ib + 1) * FB
                            ],
                            in_=o_st,
                        )

    nc.compile()
    return nc


def _prep_inputs(x, cos, sin, Wq, Wk, Wv):
    """Host-side sharding/layout prep. Returns in_maps for 8 cores."""
    x = np.asarray(x, dtype=np.float32)
    cos = np.asarray(cos, dtype=np.float32)
    sin = np.asarray(sin, dtype=np.float32)

    def w_panels(w):
        # W.T [din, dout] -> [dc, p_din, k_din, c_dout] with d = k*128+p
        wt = np.ascontiguousarray(np.asarray(w, dtype=np.float32).T).astype(BF16)
        return np.ascontiguousarray(
            wt.reshape(KD, P, KD, P).transpose(2, 1, 0, 3)
        )

    wq_p = w_panels(Wq)
    wk_p = w_panels(Wk)
    # Wv.T [din, dout] -> [p, k, dout]
    wv_p = np.ascontiguousarray(
        np.asarray(Wv, dtype=np.float32).T.astype(BF16).reshape(KD, P, D).transpose(1, 0, 2)
    )

    in_maps = []
    for c in range(N_CORES):
        b, h = divmod(c, 2)
        rows = slice(h * NOWN, (h + 1) * NOWN)
        xt = np.ascontiguousarray(
            x[b, rows, :].T.astype(BF16).reshape(KD, P, NOWN).transpose(1, 0, 2)
        )
        cos_t = np.ascontiguousarray(cos[rows].T.astype(BF16).reshape(KD, P, NOWN))
        sin_t = np.ascontiguousarray(sin[rows].T.astype(BF16).reshape(KD, P, NOWN))
        j = np.arange(S, dtype=np.int64)[:, None]
        i = (np.arange(NOWN, dtype=np.int64) + h * NOWN)[None, :]
        mask_t = (j <= i).astype(BF16).reshape(NCJ, P, NOWN)
        in_maps.append(
            {
                "x_t": xt,
                "wq": wq_p,
                "wk": wk_p,
                "wv": wv_p,
                "cos_t": cos_t,
                "sin_t": sin_t,
                "mask_t": mask_t,
            }
        )
    return in_maps


def _run(in_maps, trace=False, tmpdir=None):
    from concourse.bass_utils import run_bass_kernel_spmd

    if "nc" not in _CACHE:
        _CACHE["nc"] = _build()
    nc = _CACHE["nc"]
    return run_bass_kernel_spmd(
        nc, in_maps, list(range(N_CORES)), trace=trace, tmpdir=tmpdir
    )


def kernel(x, cos, sin, Wq, Wk, Wv):
    in_maps = _prep_inputs(x, cos, sin, Wq, Wk, Wv)
    res = _run(in_maps, trace=False)
    out = np.empty((B, S, D), dtype=np.float32)
    for c in range(N_CORES):
        b, h = divmod(c, 2)
        out[b, h * NOWN : (h + 1) * NOWN, :] = res.results[c]["out"].T
    return out


# revision 46
# speedup vs baseline: 1.1670x; 1.0363x over previous
"""Trainium2 Bass kernel: single-head attention with RoPE and the reference's
multiplicative causal mask (masked logits stay 0 -> exp(0)=1, dense attention).

Sharding: 8 cores = 4 batches x 2 sequence-halves. Each core projects Q/K/V
for its 1024 rows (bf16 matmuls, fp32 PSUM), applies RoPE on-chip, all-gathers
roped K and V within the 2-core pair, then computes attention for its rows.
Output is O^T per core; the host transposes and reassembles.
"""

import sys

for _p in ("/opt/trn_rl_repo", "/root/.axon_site/_ro/trn_rl_repo"):
    if _p not in sys.path:
        sys.path.append(_p)

import math

import ml_dtypes
import numpy as np

BF16 = ml_dtypes.bfloat16

B, S, D = 4, 2048, 2048
NOWN = 1024  # query rows per core
P = 128  # partitions
KD = D // P  # 16 feature chunks
NCJ = S // P  # 16 key chunks
N_CORES = 8
PAIRS = [[0, 1], [2, 3], [4, 5], [6, 7]]
FB = 512  # matmul moving free-dim block
NB = NOWN // FB  # 2 blocks of own rows
SCALE = 1.0 / math.sqrt(S)  # reference scales by sqrt(seq_len), not sqrt(D)

_CACHE = {}


def _build():
    import concourse.bass as bass  # noqa: F401
    import concourse.tile as tile
    from concourse import bacc, mybir

    f32 = mybir.dt.float32
    bf16 = mybir.dt.bfloat16

    nc = bacc.Bacc(
        "TRN2", target_bir_lowering=False, debug=False, num_devices=N_CORES
    )

    x_ext = nc.dram_tensor("x_t", [P, KD, NOWN], bf16, kind="ExternalInput").ap()
    wq_ext = nc.dram_tensor("wq", [KD, P, KD, P], bf16, kind="ExternalInput").ap()
    wk_ext = nc.dram_tensor("wk", [KD, P, KD, P], bf16, kind="ExternalInput").ap()
    wv_ext = nc.dram_tensor("wv", [P, KD, D], bf16, kind="ExternalInput").ap()
    cos_ext = nc.dram_tensor("cos_t", [KD, P, NOWN], bf16, kind="ExternalInput").ap()
    sin_ext = nc.dram_tensor("sin_t", [KD, P, NOWN], bf16, kind="ExternalInput").ap()
    mask_ext = nc.dram_tensor("mask_t", [NCJ, P, FB], bf16, kind="ExternalInput").ap()
    out_ext = nc.dram_tensor("out", [D, NOWN], f32, kind="ExternalOutput").ap()

    with tile.TileContext(nc) as tc:
        with (
            tc.tile_pool(name="dram", bufs=1, space="DRAM") as dram,
            tc.tile_pool(name="psum", bufs=6, space="PSUM") as psum,
            tc.tile_pool(name="dnsum", bufs=1, space="PSUM") as dnsum,
            tc.tile_pool(name="persist", bufs=1) as persist,
            tc.tile_pool(name="tmp", bufs=4) as tmp,
            tc.tile_pool(name="csp", bufs=4) as csp,
            tc.tile_pool(name="strm", bufs=8) as strm,
        ):
            kt_local = dram.tile([NCJ // 2, P, KD, P], bf16)
            v_local = dram.tile([NCJ // 2, P, D], bf16)
            # gathered tensors, split in halves so each 2MB gather can launch
            # as soon as its half is produced (pipelines with compute)
            kt_ga = dram.tile([2, 4, P, KD, P], bf16)
            kt_gb = dram.tile([2, 4, P, KD, P], bf16)
            v_ga = dram.tile([2, 4, P, D], bf16)
            v_gb = dram.tile([2, 4, P, D], bf16)

            def kt_g(jc):
                h2, jcl = jc // 8, jc % 8
                return (kt_ga if jcl < 4 else kt_gb)[h2, jcl % 4]

            def v_g(jc):
                h2, jcl = jc // 8, jc % 8
                return (v_ga if jcl < 4 else v_gb)[h2, jcl % 4]

            ones_col = persist.tile([P, 1], bf16)
            nc.vector.memset(ones_col, 1.0)
            ones_row = persist.tile([1, P], f32)
            nc.vector.memset(ones_row, 1.0)



            # x in 8 independent tiles so the chunk DMAs run in parallel
            # (DMAs into one tile serialize on its semaphore)
            x_pool = tc.alloc_tile_pool(name="x_pool", bufs=1)
            x_ts = [
                x_pool.tile([P, 2, NOWN], bf16, name=f"x_sb{i}") for i in range(8)
            ]
            x_dma_engines = [nc.sync, nc.scalar, nc.gpsimd, nc.sync]
            for kg in range(8):
                x_dma_engines[kg % 4].dma_start(
                    out=x_ts[kg], in_=x_ext[:, kg * 2 : (kg + 1) * 2, :]
                )

            def x_ref(k):
                return x_ts[k // 2][:, k % 2, :]

            # wv in 4 independent tiles; loaded mid-K-phase so it overlaps K
            # compute without stealing startup DMA bandwidth from x
            wv_pool = tc.alloc_tile_pool(name="wv_pool", bufs=1)
            wv_ts = [
                wv_pool.tile([P, 4, D], bf16, name=f"wv_sb{i}") for i in range(4)
            ]

            def wv_ref(k):
                return wv_ts[k // 4][:, k % 4, :]

            def emit_wv_load():
                for kg in range(4):
                    eng = nc.scalar if kg % 2 == 0 else nc.gpsimd
                    eng.dma_start(
                        out=wv_ts[kg], in_=wv_ext[:, kg * 4 : (kg + 1) * 4, :]
                    )

            qt_sb = persist.tile([P, KD, NOWN], bf16)

            def load_panels(wpool, w_ext, dlow, nb):
                dhigh = dlow + KD // 2
                w_lo = wpool.tile([P, KD, P], bf16, tag="wp", name=f"wlo{dlow}{nb}")
                nc.sync.dma_start(out=w_lo, in_=w_ext[dlow])
                w_hi = wpool.tile([P, KD, P], bf16, tag="wp", name=f"whi{dlow}{nb}")
                nc.sync.dma_start(out=w_hi, in_=w_ext[dhigh])
                return w_lo, w_hi

            def rope_pair(panels, dlow, nb, cs_tiles, out_ap, post):
                """One (dlow, nb) unit: two projections + rope."""
                dhigh = dlow + KD // 2
                sl = slice(nb * FB, (nb + 1) * FB)
                cos_t, sin_t, cos_h, sin_h = cs_tiles
                w_lo, w_hi = panels
                ps_lo = psum.tile([P, FB], f32, tag="ps", name=f"plo{dlow}{nb}")
                for k in range(KD):
                    nc.tensor.matmul(
                        ps_lo,
                        lhsT=w_lo[:, k, :],
                        rhs=x_ref(k)[:, sl],
                        start=(k == 0),
                        stop=(k == KD - 1),
                    )
                ps_hi = psum.tile([P, FB], f32, tag="ps", name=f"phi{dlow}{nb}")
                for k in range(KD):
                    nc.tensor.matmul(
                        ps_hi,
                        lhsT=w_hi[:, k, :],
                        rhs=x_ref(k)[:, sl],
                        start=(k == 0),
                        stop=(k == KD - 1),
                    )
                # rope low half: out = lo*cos_l - hi*sin_l
                t1 = tmp.tile([P, FB], f32, tag="t", name=f"t1{dlow}{nb}")
                nc.vector.tensor_mul(t1, ps_lo, cos_t)
                t2 = tmp.tile([P, FB], f32, tag="t", name=f"t2{dlow}{nb}")
                nc.vector.tensor_mul(t2, ps_hi, sin_t)
                o_lo = out_ap(dlow, nb)
                nc.vector.tensor_sub(o_lo, t1, t2)
                if post is not None:
                    post(dlow, nb, o_lo)
                # rope high half: out = hi*cos_h + lo*sin_h
                t3 = tmp.tile([P, FB], f32, tag="t", name=f"t3{dlow}{nb}")
                nc.vector.tensor_mul(t3, ps_hi, cos_h)
                t4 = tmp.tile([P, FB], f32, tag="t", name=f"t4{dlow}{nb}")
                nc.vector.tensor_mul(t4, ps_lo, sin_h)
                o_hi = out_ap(dhigh, nb)
                nc.vector.tensor_add(o_hi, t3, t4)
                if post is not None:
                    post(dhigh, nb, o_hi)

            def cs_load(dlow, sl):
                dhigh = dlow + KD // 2
                tiles = []
                for name, src in (
                    ("ct", cos_ext[dlow]),
                    ("st", sin_ext[dlow]),
                    ("ch", cos_ext[dhigh]),
                    ("sh", sin_ext[dhigh]),
                ):
                    t = csp.tile(
                        [P, sl.stop - sl.start], bf16, tag="cs", name=f"cs_{name}"
                    )
                    nc.scalar.dma_start(out=t, in_=src[:, sl])
                    tiles.append(t)
                return tiles

            def project_rope(w_ext, wpool, out_ap, post=None):
                """dlow-outer order (used for Q); panels loaded once per dlow."""
                for dlow in range(KD // 2):
                    cs_full = cs_load(dlow, slice(0, NOWN))
                    panels = load_panels(wpool, w_ext, dlow, 0)
                    for nb in range(NB):
                        sl = slice(nb * FB, (nb + 1) * FB)
                        cs_tiles = [t[:, sl] for t in cs_full]
                        rope_pair(panels, dlow, nb, cs_tiles, out_ap, post)

            # ---- K projection + rope -> kt_local -> split AllGathers ----
            def k_out(dc, nb):
                t = strm.tile([P, FB], bf16, tag="ro", name=f"kt_{dc}_{nb}")
                return t

            def k_post(dc, nb, t):
                for jj in range(FB // P):
                    nc.gpsimd.dma_start(
                        out=kt_local[nb * 4 + jj][:, dc, :],
                        in_=t[:, jj * P : (jj + 1) * P],
                    )

            def emit_kt_gather(half, out_t):
                nc.gpsimd.collective_compute(
                    "AllGather",
                    mybir.AluOpType.bypass,
                    replica_groups=PAIRS,
                    ins=[kt_local[half * 4 : (half + 1) * 4].opt()],
                    outs=[out_t.opt()],
                )

            # nb-outer so each half of kt_local completes early and its
            # gather pipelines with the rest of the projections
            with tc.tile_pool(name="wk_pool", bufs=6) as wkp:
                for nb in range(NB):
                    for dlow in range(KD // 2):
                        if nb == 0 and dlow == 1:
                            emit_wv_load()
                        cs_tiles = cs_load(dlow, slice(nb * FB, (nb + 1) * FB))
                        panels = load_panels(wkp, wk_ext, dlow, nb)
                        rope_pair(panels, dlow, nb, cs_tiles, k_out, k_post)
                    emit_kt_gather(nb, kt_ga if nb == 0 else kt_gb)

            # ---- V projection (natural layout) -> v_local -> split gathers ----
            for ncc in range(NCJ // 2):
                for wb in range(D // FB):
                    ps_v = psum.tile([P, FB], f32, tag="ps")
                    for k in range(KD):
                        nc.tensor.matmul(
                            ps_v,
                            lhsT=x_ref(k)[:, ncc * P : (ncc + 1) * P],
                            rhs=wv_ref(k)[:, wb * FB : (wb + 1) * FB],
                            start=(k == 0),
                            stop=(k == KD - 1),
                        )
                    v_t = strm.tile([P, FB], bf16, tag="vo")
                    nc.vector.tensor_copy(v_t, ps_v)
                    nc.gpsimd.dma_start(
                        out=v_local[ncc][:, wb * FB : (wb + 1) * FB], in_=v_t
                    )
                if ncc == 3 or ncc == 7:
                    half = ncc // 4
                    nc.gpsimd.collective_compute(
                        "AllGather",
                        mybir.AluOpType.bypass,
                        replica_groups=PAIRS,
                        ins=[v_local[half * 4 : (half + 1) * 4].opt()],
                        outs=[(v_ga if half == 0 else v_gb).opt()],
                    )
            wv_pool.release()

            # ---- Q projection + rope (overlaps the collectives) ----
            def q_out(dc, nb):
                return qt_sb[:, dc, nb * FB : (nb + 1) * FB]

            with tc.tile_pool(name="wq_pool", bufs=6) as wqp:
                project_rope(wq_ext, wqp, q_out)
            x_pool.release()

            # ---- Attention ----
            with (
                tc.tile_pool(name="v2_pool", bufs=1) as v2p,
                tc.tile_pool(name="pt_pool", bufs=1) as ptp,
                tc.tile_pool(name="slab", bufs=6) as slab,
                tc.tile_pool(name="mskp", bufs=3) as mskp,
                tc.tile_pool(name="outp", bufs=4) as outp,
                tc.tile_pool(name="smallp", bufs=2) as smallp,
            ):
                v2_sb = v2p.tile([P, NCJ, D], bf16)
                for jc in range(NCJ):
                    nc.gpsimd.dma_start(out=v2_sb[:, jc, :], in_=v_g(jc))

                pt_sb = [
                    ptp.tile([P, NCJ, FB], bf16, name=f"pt_sb{ib}")
                    for ib in range(NB)
                ]

                # With interleaved-row sharding the mask tile classes are the
                # same on every core:
                #   jc%8 < 4: (jc, ib0)=mixed(mask), (jc, ib1)=fully unmasked
                #   jc%8 >= 4: (jc, ib0)=fully masked (P==1), (jc, ib1)=mixed
                # Fully-masked tiles skip the matmul chain entirely; their
                # exp(0)=1 entries are memset into PT.
                for jc in range(NCJ):
                    if jc % 8 >= 4:
                        nc.vector.memset(pt_sb[0][:, jc, :], 1.0)

                def s_tile(jc, ib, kt_slab, msk):
                    sl = slice(ib * FB, (ib + 1) * FB)
                    ps_s = psum.tile([P, FB], f32, tag="ps", name=f"ps_s{jc}{ib}")
                    for k in range(KD):
                        nc.tensor.matmul(
                            ps_s,
                            lhsT=kt_slab[:, k, :],
                            rhs=qt_sb[:, k, sl],
                            start=(k == 0),
                            stop=(k == KD - 1),
                        )
                    if msk is not None:
                        tm = tmp.tile([P, FB], f32, tag="t", name=f"tm{jc}{ib}")
                        nc.vector.tensor_mul(tm, ps_s, msk)
                        src = tm
                    else:
                        src = ps_s
                    nc.scalar.activation(
                        out=pt_sb[ib][:, jc, :],
                        in_=src,
                        func=mybir.ActivationFunctionType.Exp,
                        scale=SCALE,
                    )

                for jc in range(NCJ):
                    kt_slab = slab.tile([P, KD, P], bf16, tag="slab")
                    nc.sync.dma_start(out=kt_slab, in_=kt_g(jc))
                    msk = mskp.tile([P, FB], bf16, tag="m")
                    nc.scalar.dma_start(out=msk, in_=mask_ext[jc])
                    if jc % 8 < 4:
                        s_tile(jc, 0, kt_slab, msk)  # mixed
                        s_tile(jc, 1, kt_slab, None)  # fully unmasked
                    else:
                        s_tile(jc, 1, kt_slab, msk)  # mixed; ib0 skipped

                # denominators + reciprocal broadcasts
                rbs = []
                for ib in range(NB):
                    ps_d = dnsum.tile([1, FB], f32, tag="dn")
                    for jc in range(NCJ):
                        nc.tensor.matmul(
                            ps_d,
                            lhsT=ones_col,
                            rhs=pt_sb[ib][:, jc, :],
                            start=(jc == 0),
                            stop=(jc == NCJ - 1),
                        )
                    recip = smallp.tile([1, FB], f32, tag="rc")
                    nc.vector.reciprocal(recip, ps_d)
                    ps_rb = dnsum.tile([P, FB], f32, tag="rb")
                    nc.tensor.matmul(
                        ps_rb, lhsT=ones_row, rhs=recip, start=True, stop=True
                    )
                    rb = smallp.tile([P, FB], f32, tag="rbs")
                    nc.vector.tensor_copy(rb, ps_rb)
                    rbs.append(rb)

                # O^T = V^T @ P^T, scaled by 1/denom
                for ib in range(NB):
                    rb = rbs[ib]
                    for dc in range(KD):
                        ps_o = psum.tile([P, FB], f32, tag="ps")
                        for jc in range(NCJ):
                            nc.tensor.matmul(
                                ps_o,
                                lhsT=v2_sb[:, jc, dc * P : (dc + 1) * P],
                                rhs=pt_sb[ib][:, jc, :],
                                start=(jc == 0),
                                stop=(jc == NCJ - 1),
                            )
                        o_st = outp.tile([P, FB], f32, tag="o")
                        nc.vector.tensor_mul(o_st, ps_o, rb)
                        nc.gpsimd.dma_start(
                            out=out_ext[
                                dc * P : (dc + 1) * P, ib * FB : (ib + 1) * FB
                            ],
                            in_=o_st,
                        )

    nc.compile()
    return nc


def _prep_inputs(x, cos, sin, Wq, Wk, Wv):
    """Host-side sharding/layout prep. Returns in_maps for 8 cores."""
    x = np.asarray(x, dtype=np.float32)
    cos = np.asarray(cos, dtype=np.float32)
    sin = np.asarray(sin, dtype=np.float32)

    def w_panels(w):
        # W.T [din, dout] -> [dc, p_din, k_din, c_dout] with d = k*128+p
        wt = np.ascontiguousarray(np.asarray(w, dtype=np.float32).T).astype(BF16)
        return np.ascontiguousarray(
            wt.reshape(KD, P, KD, P).transpose(2, 1, 0, 3)
        )

    wq_p = w_panels(Wq)
    wk_p = w_panels(Wk)
    # Wv.T [din, dout] -> [p, k, dout]
    wv_p = np.ascontiguousarray(
        np.asarray(Wv, dtype=np.float32).T.astype(BF16).reshape(KD, P, D).transpose(1, 0, 2)
    )

    # global row index of gathered slot s: pair rank h2 = s // NOWN owns the
    # rows with parity h2, so j_global(s) = 2*(s % NOWN) + h2
    slot = np.arange(S, dtype=np.int64)
    j_global = 2 * (slot % NOWN) + slot // NOWN

    in_maps = []
    for c in range(N_CORES):
        b, h = divmod(c, 2)
        rows = slice(h, None, 2)  # interleaved rows: h, h+2, h+4, ...
        xt = np.ascontiguousarray(
            x[b, rows, :].T.astype(BF16).reshape(KD, P, NOWN).transpose(1, 0, 2)
        )
        cos_t = np.ascontiguousarray(cos[rows].T.astype(BF16).reshape(KD, P, NOWN))
        sin_t = np.ascontiguousarray(sin[rows].T.astype(BF16).reshape(KD, P, NOWN))
        i_global = 2 * np.arange(NOWN, dtype=np.int64) + h
        # per jc, only the "mixed" i-half needs mask data
        mask_t = np.empty((NCJ, P, FB), dtype=BF16)
        for jc in range(NCJ):
            ib = 0 if jc % 8 < 4 else 1
            jg = j_global[jc * P : (jc + 1) * P][:, None]
            ig = i_global[ib * FB : (ib + 1) * FB][None, :]
            mask_t[jc] = (jg <= ig).astype(BF16)
        in_maps.append(
            {
                "x_t": xt,
                "wq": wq_p,
                "wk": wk_p,
                "wv": wv_p,
                "cos_t": cos_t,
                "sin_t": sin_t,
                "mask_t": mask_t,
            }
        )
    return in_maps


def _run(in_maps, trace=False, tmpdir=None):
    from concourse.bass_utils import run_bass_kernel_spmd

    if "nc" not in _CACHE:
        _CACHE["nc"] = _build()
    nc = _CACHE["nc"]
    return run_bass_kernel_spmd(
        nc, in_maps, list(range(N_CORES)), trace=trace, tmpdir=tmpdir
    )


def kernel(x, cos, sin, Wq, Wk, Wv):
    in_maps = _prep_inputs(x, cos, sin, Wq, Wk, Wv)
    res = _run(in_maps, trace=False)
    out = np.empty((B, S, D), dtype=np.float32)
    for c in range(N_CORES):
        b, h = divmod(c, 2)
        out[b, h::2, :] = res.results[c]["out"].T
    return out


# revision 49
# speedup vs baseline: 1.2169x; 1.0427x over previous
"""Trainium2 Bass kernel: single-head attention with RoPE and the reference's
multiplicative causal mask (masked logits stay 0 -> exp(0)=1, dense attention).

Sharding: 8 cores = 4 batches x 2 sequence-halves. Each core projects Q/K/V
for its 1024 rows (bf16 matmuls, fp32 PSUM), applies RoPE on-chip, all-gathers
roped K and V within the 2-core pair, then computes attention for its rows.
Output is O^T per core; the host transposes and reassembles.
"""

import sys

for _p in ("/opt/trn_rl_repo", "/root/.axon_site/_ro/trn_rl_repo"):
    if _p not in sys.path:
        sys.path.append(_p)

import math

import ml_dtypes
import numpy as np

BF16 = ml_dtypes.bfloat16

B, S, D = 4, 2048, 2048
NOWN = 1024  # query rows per core
P = 128  # partitions
KD = D // P  # 16 feature chunks
NCJ = S // P  # 16 key chunks
N_CORES = 8
PAIRS = [[0, 1], [2, 3], [4, 5], [6, 7]]
FB = 512  # matmul moving free-dim block
NB = NOWN // FB  # 2 blocks of own rows
SCALE = 1.0 / math.sqrt(S)  # reference scales by sqrt(seq_len), not sqrt(D)

_CACHE = {}


def _build():
    import concourse.bass as bass  # noqa: F401
    import concourse.tile as tile
    from concourse import bacc, mybir

    f32 = mybir.dt.float32
    bf16 = mybir.dt.bfloat16

    nc = bacc.Bacc(
        "TRN2", target_bir_lowering=False, debug=False, num_devices=N_CORES
    )

    x_ext = nc.dram_tensor("x_t", [P, KD, NOWN], bf16, kind="ExternalInput").ap()
    wq_ext = nc.dram_tensor("wq", [KD, P, KD, P], bf16, kind="ExternalInput").ap()
    wk_ext = nc.dram_tensor("wk", [KD, P, KD, P], bf16, kind="ExternalInput").ap()
    wv_ext = nc.dram_tensor("wv", [P, KD, D], bf16, kind="ExternalInput").ap()
    cos_ext = nc.dram_tensor("cos_t", [KD, P, NOWN], bf16, kind="ExternalInput").ap()
    sin_ext = nc.dram_tensor("sin_t", [KD, P, NOWN], bf16, kind="ExternalInput").ap()
    mask_ext = nc.dram_tensor("mask_t", [NCJ, P, FB], bf16, kind="ExternalInput").ap()
    out_ext = nc.dram_tensor("out", [D, NOWN], f32, kind="ExternalOutput").ap()

    with tile.TileContext(nc) as tc:
        with (
            tc.tile_pool(name="dram", bufs=1, space="DRAM") as dram,
            tc.tile_pool(name="psum", bufs=5, space="PSUM") as psum,
            tc.tile_pool(name="dnsum", bufs=1, space="PSUM") as dnsum,
            tc.tile_pool(name="persist", bufs=1) as persist,
            tc.tile_pool(name="tmp", bufs=4) as tmp,
            tc.tile_pool(name="csp", bufs=4) as csp,
            tc.tile_pool(name="strm", bufs=8) as strm,
        ):
            kt_local = dram.tile([NCJ // 2, P, KD, P], bf16)
            v_local = dram.tile([NCJ // 2, P, D], bf16)
            # gathered tensors, split in halves so each 2MB gather can launch
            # as soon as its half is produced (pipelines with compute)
            kt_ga = dram.tile([2, 4, P, KD, P], bf16)
            kt_gb = dram.tile([2, 4, P, KD, P], bf16)
            v_ga = dram.tile([2, 4, P, D], bf16)
            v_gb = dram.tile([2, 4, P, D], bf16)

            def kt_g(jc):
                h2, jcl = jc // 8, jc % 8
                return (kt_ga if jcl < 4 else kt_gb)[h2, jcl % 4]

            def v_g(jc):
                h2, jcl = jc // 8, jc % 8
                return (v_ga if jcl < 4 else v_gb)[h2, jcl % 4]

            ones_col = persist.tile([P, 1], bf16)
            nc.vector.memset(ones_col, 1.0)
            ones_row = persist.tile([1, P], f32)
            nc.vector.memset(ones_row, 1.0)



            # x in 8 independent tiles so the chunk DMAs run in parallel
            # (DMAs into one tile serialize on its semaphore)
            x_pool = tc.alloc_tile_pool(name="x_pool", bufs=1)
            x_ts = [
                x_pool.tile([P, 2, NOWN], bf16, name=f"x_sb{i}") for i in range(8)
            ]
            x_dma_engines = [nc.sync, nc.scalar, nc.gpsimd, nc.sync]
            for kg in range(8):
                x_dma_engines[kg % 4].dma_start(
                    out=x_ts[kg], in_=x_ext[:, kg * 2 : (kg + 1) * 2, :]
                )

            def x_ref(k):
                return x_ts[k // 2][:, k % 2, :]

            # wv in 4 independent tiles; loaded mid-K-phase so it overlaps K
            # compute without stealing startup DMA bandwidth from x
            wv_pool = tc.alloc_tile_pool(name="wv_pool", bufs=1)
            wv_ts = [
                wv_pool.tile([P, 4, D], bf16, name=f"wv_sb{i}") for i in range(4)
            ]

            def wv_ref(k):
                return wv_ts[k // 4][:, k % 4, :]

            def emit_wv_load():
                for kg in range(4):
                    eng = nc.scalar if kg % 2 == 0 else nc.gpsimd
                    eng.dma_start(
                        out=wv_ts[kg], in_=wv_ext[:, kg * 4 : (kg + 1) * 4, :]
                    )

            qt_sb = persist.tile([P, KD, NOWN], bf16)

            def load_panels(wpool, w_ext, dlow, nb):
                dhigh = dlow + KD // 2
                w_lo = wpool.tile([P, KD, P], bf16, tag="wp", name=f"wlo{dlow}{nb}")
                nc.sync.dma_start(out=w_lo, in_=w_ext[dlow])
                w_hi = wpool.tile([P, KD, P], bf16, tag="wp", name=f"whi{dlow}{nb}")
                nc.sync.dma_start(out=w_hi, in_=w_ext[dhigh])
                return w_lo, w_hi

            def rope_pair(panels, dlow, nb, cs_tiles, out_ap, post):
                """One (dlow, nb) unit: two projections + rope."""
                dhigh = dlow + KD // 2
                sl = slice(nb * FB, (nb + 1) * FB)
                cos_t, sin_t, cos_h, sin_h = cs_tiles
                w_lo, w_hi = panels
                ps_lo = psum.tile([P, FB], f32, tag="ps", name=f"plo{dlow}{nb}")
                for k in range(KD):
                    nc.tensor.matmul(
                        ps_lo,
                        lhsT=w_lo[:, k, :],
                        rhs=x_ref(k)[:, sl],
                        start=(k == 0),
                        stop=(k == KD - 1),
                    )
                ps_hi = psum.tile([P, FB], f32, tag="ps", name=f"phi{dlow}{nb}")
                for k in range(KD):
                    nc.tensor.matmul(
                        ps_hi,
                        lhsT=w_hi[:, k, :],
                        rhs=x_ref(k)[:, sl],
                        start=(k == 0),
                        stop=(k == KD - 1),
                    )
                # rope low half: out = lo*cos_l - hi*sin_l
                t1 = tmp.tile([P, FB], f32, tag="t", name=f"t1{dlow}{nb}")
                nc.vector.tensor_mul(t1, ps_lo, cos_t)
                t2 = tmp.tile([P, FB], f32, tag="t", name=f"t2{dlow}{nb}")
                nc.vector.tensor_mul(t2, ps_hi, sin_t)
                o_lo = out_ap(dlow, nb)
                nc.vector.tensor_sub(o_lo, t1, t2)
                if post is not None:
                    post(dlow, nb, o_lo)
                # rope high half: out = hi*cos_h + lo*sin_h
                t3 = tmp.tile([P, FB], f32, tag="t", name=f"t3{dlow}{nb}")
                nc.vector.tensor_mul(t3, ps_hi, cos_h)
                t4 = tmp.tile([P, FB], f32, tag="t", name=f"t4{dlow}{nb}")
                nc.vector.tensor_mul(t4, ps_lo, sin_h)
                o_hi = out_ap(dhigh, nb)
                nc.vector.tensor_add(o_hi, t3, t4)
                if post is not None:
                    post(dhigh, nb, o_hi)

            def cs_load(dlow, sl):
                dhigh = dlow + KD // 2
                tiles = []
                for name, src in (
                    ("ct", cos_ext[dlow]),
                    ("st", sin_ext[dlow]),
                    ("ch", cos_ext[dhigh]),
                    ("sh", sin_ext[dhigh]),
                ):
                    t = csp.tile(
                        [P, sl.stop - sl.start], bf16, tag="cs", name=f"cs_{name}"
                    )
                    nc.scalar.dma_start(out=t, in_=src[:, sl])
                    tiles.append(t)
                return tiles

            def project_rope(w_ext, wpool, out_ap, post=None):
                """dlow-outer order (used for Q); panels loaded once per dlow."""
                for dlow in range(KD // 2):
                    cs_full = cs_load(dlow, slice(0, NOWN))
                    panels = load_panels(wpool, w_ext, dlow, 0)
                    for nb in range(NB):
                        sl = slice(nb * FB, (nb + 1) * FB)
                        cs_tiles = [t[:, sl] for t in cs_full]
                        rope_pair(panels, dlow, nb, cs_tiles, out_ap, post)

            # ---- K projection + rope -> kt_local -> split AllGathers ----
            def k_out(dc, nb):
                t = strm.tile([P, FB], bf16, tag="ro", name=f"kt_{dc}_{nb}")
                return t

            def k_post(dc, nb, t):
                for jj in range(FB // P):
                    nc.gpsimd.dma_start(
                        out=kt_local[nb * 4 + jj][:, dc, :],
                        in_=t[:, jj * P : (jj + 1) * P],
                    )

            def emit_kt_gather(half, out_t):
                nc.gpsimd.collective_compute(
                    "AllGather",
                    mybir.AluOpType.bypass,
                    replica_groups=PAIRS,
                    ins=[kt_local[half * 4 : (half + 1) * 4].opt()],
                    outs=[out_t.opt()],
                )

            # nb-outer so each half of kt_local completes early and its
            # gather pipelines with the rest of the projections
            with tc.tile_pool(name="wk_pool", bufs=6) as wkp:
                for nb in range(NB):
                    for dlow in range(KD // 2):
                        if nb == 0 and dlow == 1:
                            emit_wv_load()
                        cs_tiles = cs_load(dlow, slice(nb * FB, (nb + 1) * FB))
                        panels = load_panels(wkp, wk_ext, dlow, nb)
                        rope_pair(panels, dlow, nb, cs_tiles, k_out, k_post)
                    emit_kt_gather(nb, kt_ga if nb == 0 else kt_gb)

            # ---- V projection (natural layout) -> v_local -> split gathers ----
            for ncc in range(NCJ // 2):
                for wb in range(D // FB):
                    ps_v = psum.tile([P, FB], f32, tag="ps")
                    for k in range(KD):
                        nc.tensor.matmul(
                            ps_v,
                            lhsT=x_ref(k)[:, ncc * P : (ncc + 1) * P],
                            rhs=wv_ref(k)[:, wb * FB : (wb + 1) * FB],
                            start=(k == 0),
                            stop=(k == KD - 1),
                        )
                    v_t = strm.tile([P, FB], bf16, tag="vo")
                    nc.vector.tensor_copy(v_t, ps_v)
                    nc.gpsimd.dma_start(
                        out=v_local[ncc][:, wb * FB : (wb + 1) * FB], in_=v_t
                    )
                if ncc == 3 or ncc == 7:
                    half = ncc // 4
                    nc.gpsimd.collective_compute(
                        "AllGather",
                        mybir.AluOpType.bypass,
                        replica_groups=PAIRS,
                        ins=[v_local[half * 4 : (half + 1) * 4].opt()],
                        outs=[(v_ga if half == 0 else v_gb).opt()],
                    )
            wv_pool.release()

            # ---- Q projection + rope (overlaps the collectives) ----
            def q_out(dc, nb):
                return qt_sb[:, dc, nb * FB : (nb + 1) * FB]

            with tc.tile_pool(name="wq_pool", bufs=6) as wqp:
                project_rope(wq_ext, wqp, q_out)
            x_pool.release()

            # ---- Attention ----
            with (
                tc.tile_pool(name="v2_pool", bufs=1) as v2p,
                tc.tile_pool(name="pt_pool", bufs=1) as ptp,
                tc.tile_pool(name="slab", bufs=6) as slab,
                tc.tile_pool(name="mskp", bufs=3) as mskp,
                tc.tile_pool(name="outp", bufs=4) as outp,
                tc.tile_pool(name="smallp", bufs=2) as smallp,
            ):
                v2_sb = v2p.tile([P, NCJ, D], bf16)
                for jc in range(NCJ):
                    nc.gpsimd.dma_start(out=v2_sb[:, jc, :], in_=v_g(jc))

                pt_sb = [
                    ptp.tile([P, NCJ, FB], bf16, name=f"pt_sb{ib}")
                    for ib in range(NB)
                ]

                # With interleaved-row sharding the mask tile classes are the
                # same on every core:
                #   jc%8 < 4: (jc, ib0)=mixed(mask), (jc, ib1)=fully unmasked
                #   jc%8 >= 4: (jc, ib0)=fully masked (P==1), (jc, ib1)=mixed
                # Fully-masked tiles skip the matmul chain entirely; their
                # exp(0)=1 entries are memset into PT.
                def s_tile(jc, ib, kt_slab, msk):
                    sl = slice(ib * FB, (ib + 1) * FB)
                    ps_s = psum.tile([P, FB], f32, tag="ps", name=f"ps_s{jc}{ib}")
                    for k in range(KD):
                        nc.tensor.matmul(
                            ps_s,
                            lhsT=kt_slab[:, k, :],
                            rhs=qt_sb[:, k, sl],
                            start=(k == 0),
                            stop=(k == KD - 1),
                        )
                    if msk is not None:
                        tm = tmp.tile([P, FB], f32, tag="t", name=f"tm{jc}{ib}")
                        nc.vector.tensor_mul(tm, ps_s, msk)
                        src = tm
                    else:
                        src = ps_s
                    nc.scalar.activation(
                        out=pt_sb[ib][:, jc, :],
                        in_=src,
                        func=mybir.ActivationFunctionType.Exp,
                        scale=SCALE,
                    )

                for jc in range(NCJ):
                    kt_slab = slab.tile([P, KD, P], bf16, tag="slab")
                    nc.sync.dma_start(out=kt_slab, in_=kt_g(jc))
                    msk = mskp.tile([P, FB], bf16, tag="m")
                    nc.scalar.dma_start(out=msk, in_=mask_ext[jc])
                    if jc % 8 < 4:
                        s_tile(jc, 0, kt_slab, msk)  # mixed
                        s_tile(jc, 1, kt_slab, None)  # fully unmasked
                    else:
                        s_tile(jc, 1, kt_slab, msk)  # mixed; ib0 skipped

                # j-chunks whose PT(ib0) is identically 1 (fully masked);
                # their PV contribution per row is a constant vector:
                # onesum[dout] = sum over those chunks' V rows.
                SKIP0 = [jc for jc in range(NCJ) if jc % 8 >= 4]
                COMP0 = [jc for jc in range(NCJ) if jc % 8 < 4]

                ps_os = dnsum.tile([P, KD], f32, tag="os")
                for dc in range(KD):
                    for idx, jc in enumerate(SKIP0):
                        nc.tensor.matmul(
                            ps_os[:, dc : dc + 1],
                            lhsT=v2_sb[:, jc, dc * P : (dc + 1) * P],
                            rhs=ones_col,
                            start=(idx == 0),
                            stop=(idx == len(SKIP0) - 1),
                        )
                onesum = smallp.tile([P, KD], f32, tag="os_sb")
                nc.vector.tensor_copy(onesum, ps_os)

                # denominators + reciprocal broadcasts. ib0's skipped chunks
                # contribute exactly len(SKIP0)*P ones.
                rbs = []
                for ib in range(NB):
                    jcs = COMP0 if ib == 0 else list(range(NCJ))
                    ps_d = dnsum.tile([1, FB], f32, tag="dn")
                    for idx, jc in enumerate(jcs):
                        nc.tensor.matmul(
                            ps_d,
                            lhsT=ones_col,
                            rhs=pt_sb[ib][:, jc, :],
                            start=(idx == 0),
                            stop=(idx == len(jcs) - 1),
                        )
                    recip = smallp.tile([1, FB], f32, tag="rc")
                    if ib == 0:
                        dfix = smallp.tile([1, FB], f32, tag="dfix")
                        nc.vector.tensor_scalar_add(
                            dfix, ps_d, float(len(SKIP0) * P)
                        )
                        nc.vector.reciprocal(recip, dfix)
                    else:
                        nc.vector.reciprocal(recip, ps_d)
                    ps_rb = dnsum.tile([P, FB], f32, tag="rb")
                    nc.tensor.matmul(
                        ps_rb, lhsT=ones_row, rhs=recip, start=True, stop=True
                    )
                    rb = smallp.tile([P, FB], f32, tag="rbs")
                    nc.vector.tensor_copy(rb, ps_rb)
                    rbs.append(rb)

                # O^T = V^T @ P^T, scaled by 1/denom; ib0 adds the constant
                # onesum correction inside the scale op
                for ib in range(NB):
                    rb = rbs[ib]
                    jcs = COMP0 if ib == 0 else list(range(NCJ))
                    for dc in range(KD):
                        ps_o = psum.tile([P, FB], f32, tag="ps")
                        for idx, jc in enumerate(jcs):
                            nc.tensor.matmul(
                                ps_o,
                                lhsT=v2_sb[:, jc, dc * P : (dc + 1) * P],
                                rhs=pt_sb[ib][:, jc, :],
                                start=(idx == 0),
                                stop=(idx == len(jcs) - 1),
                            )
                        o_st = outp.tile([P, FB], f32, tag="o")
                        if ib == 0:
                            nc.vector.scalar_tensor_tensor(
                                out=o_st,
                                in0=ps_o,
                                scalar=onesum[:, dc : dc + 1],
                                in1=rb,
                                op0=mybir.AluOpType.add,
                                op1=mybir.AluOpType.mult,
                            )
                        else:
                            nc.vector.tensor_mul(o_st, ps_o, rb)
                        nc.gpsimd.dma_start(
                            out=out_ext[
                                dc * P : (dc + 1) * P, ib * FB : (ib + 1) * FB
                            ],
                            in_=o_st,
                        )

    nc.compile()
    return nc


def _prep_inputs(x, cos, sin, Wq, Wk, Wv):
    """Host-side sharding/layout prep. Returns in_maps for 8 cores."""
    x = np.asarray(x, dtype=np.float32)
    cos = np.asarray(cos, dtype=np.float32)
    sin = np.asarray(sin, dtype=np.float32)

    def w_panels(w):
        # W.T [din, dout] -> [dc, p_din, k_din, c_dout] with d = k*128+p
        wt = np.ascontiguousarray(np.asarray(w, dtype=np.float32).T).astype(BF16)
        return np.ascontiguousarray(
            wt.reshape(KD, P, KD, P).transpose(2, 1, 0, 3)
        )

    wq_p = w_panels(Wq)
    wk_p = w_panels(Wk)
    # Wv.T [din, dout] -> [p, k, dout]
    wv_p = np.ascontiguousarray(
        np.asarray(Wv, dtype=np.float32).T.astype(BF16).reshape(KD, P, D).transpose(1, 0, 2)
    )

    # global row index of gathered slot s: pair rank h2 = s // NOWN owns the
    # rows with parity h2, so j_global(s) = 2*(s % NOWN) + h2
    slot = np.arange(S, dtype=np.int64)
    j_global = 2 * (slot % NOWN) + slot // NOWN

    in_maps = []
    for c in range(N_CORES):
        b, h = divmod(c, 2)
        rows = slice(h, None, 2)  # interleaved rows: h, h+2, h+4, ...
        xt = np.ascontiguousarray(
            x[b, rows, :].T.astype(BF16).reshape(KD, P, NOWN).transpose(1, 0, 2)
        )
        cos_t = np.ascontiguousarray(cos[rows].T.astype(BF16).reshape(KD, P, NOWN))
        sin_t = np.ascontiguousarray(sin[rows].T.astype(BF16).reshape(KD, P, NOWN))
        i_global = 2 * np.arange(NOWN, dtype=np.int64) + h
        # per jc, only the "mixed" i-half needs mask data
        mask_t = np.empty((NCJ, P, FB), dtype=BF16)
        for jc in range(NCJ):
            ib = 0 if jc % 8 < 4 else 1
            jg = j_global[jc * P : (jc + 1) * P][:, None]
            ig = i_global[ib * FB : (ib + 1) * FB][None, :]
            mask_t[jc] = (jg <= ig).astype(BF16)
        in_maps.append(
            {
                "x_t": xt,
                "wq": wq_p,
                "wk": wk_p,
                "wv": wv_p,
                "cos_t": cos_t,
                "sin_t": sin_t,
                "mask_t": mask_t,
            }
        )
    return in_maps


def _run(in_maps, trace=False, tmpdir=None):
    from concourse.bass_utils import run_bass_kernel_spmd

    if "nc" not in _CACHE:
        _CACHE["nc"] = _build()
    nc = _CACHE["nc"]
    return run_bass_kernel_spmd(
        nc, in_maps, list(range(N_CORES)), trace=trace, tmpdir=tmpdir
    )


def kernel(x, cos, sin, Wq, Wk, Wv):
    in_maps = _prep_inputs(x, cos, sin, Wq, Wk, Wv)
    res = _run(in_maps, trace=False)
    out = np.empty((B, S, D), dtype=np.float32)
    for c in range(N_CORES):
        b, h = divmod(c, 2)
        out[b, h::2, :] = res.results[c]["out"].T
    return out
